# revision 1
# baseline (speedup 1.0000x reference)
"""Bass/Trainium2 kernel for nn_Encoder (embedding -> BiLSTM -> cross attention -> enhancement).

Sharding: data-parallel over batch, 16 items per core on 8 NeuronCores
(per the data-parallel hint; no collectives needed). Per core the A and B
sequences are stacked into 32 rows and the fwd/bwd LSTM directions run as two
interleaved dependency chains sharing the engines.

Phases per core: (1) input projections x@Wih^T+bias for both directions as
dense f32r matmuls staged to DRAM, (2) the 128-step recurrent scan — per step
and direction 16 h^T@Whh^T f32r matmuls into PSUM, per-bank DVE adds of the
staged xw, per-gate in-place activations in [g,i,f,o] order (so tanh(g)
starts after the first PSUM bank), cell/hidden elementwise, and a PE
transpose producing the next step's stationary h^T, (3) cross-attention:
PE transposes to feature-major, E/E^T f32 matmuls, row softmaxes via
Exp-with-accum, soft alignments as f32r matmuls, and the 4-way enhancement
concat streamed straight to the outputs.

float32r (full-rate fp32 PE mode, ~1e-4 matmul rel err) is used for all
large-N matmuls; elementwise math and the small-N attention logit matmuls
stay fp32.
"""

import numpy as np

V, E, H = 32000, 300, 512
BSZ, T = 128, 128
NCORES = 8
PB = BSZ // NCORES          # 16 batch items per core
RW = 2 * PB                 # 32 stacked rows (A items then B items)
RT = 2 * RW                 # 64 rows in fused fwd+bwd elementwise space
G4 = 4 * H                  # 2048 gate width
H2 = 2 * H                  # 1024 bilstm output width
KCH = [(0, 128), (128, 128), (256, 44)]   # chunks of E=300

_CACHE = {}


def _build(phases=3, scan_T=T, xwp_bufs=2, gp_bufs=2, a3_bufs=2, eps_bufs=2):
    import concourse.mybir as mybir
    import concourse.tile as tile
    from concourse import bacc
    from concourse.masks import make_identity

    F32 = mybir.dt.float32
    F32R = mybir.dt.float32r
    F16 = mybir.dt.float16
    AF = mybir.ActivationFunctionType
    ALU = mybir.AluOpType
    AX = mybir.AxisListType

    nc = bacc.Bacc("TRN2", target_bir_lowering=False, debug=False,
                   num_devices=NCORES)

    xT_d = nc.dram_tensor("xT", [E, RW * T], F32R, kind="ExternalInput")
    wih_d = {d: nc.dram_tensor(f"wihT_{d}", [E, G4], F32R, kind="ExternalInput")
             for d in "fb"}
    whh_d = {d: nc.dram_tensor(f"whhT_{d}", [H, G4], F32R, kind="ExternalInput")
             for d in "fb"}
    bias_d = {d: nc.dram_tensor(f"bias_{d}", [128, G4], F32, kind="ExternalInput")
              for d in "fb"}
    outA_d = nc.dram_tensor("outA", [PB, T, 4 * H2], F32, kind="ExternalOutput")
    outB_d = nc.dram_tensor("outB", [PB, T, 4 * H2], F32, kind="ExternalOutput")

    with tile.TileContext(nc) as tc:
        with tc.tile_pool(name="dram", bufs=1, space="DRAM") as dpool, \
             tc.tile_pool(name="const", bufs=1) as const:
            xw = {d: dpool.tile([RW, T, G4], F16, name=f"xw_{d}") for d in "fb"}
            tm = dpool.tile([RW, T, H2], F32R)
            ident = const.tile([128, 128], F32)
            make_identity(nc, ident[:])
            identr = const.tile([128, 128], F32R)
            nc.vector.tensor_copy(identr[:], ident[:])
            ident16 = const.tile([128, 128], F16)
            nc.vector.tensor_copy(ident16[:], ident[:])

            # ---------------- Phase 1: input projections ----------------
            with tc.tile_pool(name="p1w", bufs=1) as p1w, \
                 tc.tile_pool(name="p1ps", bufs=2, space="PSUM") as p1ps, \
                 tc.tile_pool(name="p1e", bufs=3) as p1e:
                xT_sb = []
                for ki, (ko, ks) in enumerate(KCH):
                    t_ = p1w.tile([ks, RW * T], F32R, tag=f"xT{ki}")
                    nc.sync.dma_start(t_[:], xT_d.ap()[ko:ko + ks, :])
                    xT_sb.append(t_)
                for d in "fb":
                    wih_sb = []
                    for ki, (ko, ks) in enumerate(KCH):
                        t_ = p1w.tile([ks, G4], F32R, tag=f"wih{d}{ki}")
                        nc.sync.dma_start(t_[:], wih_d[d].ap()[ko:ko + ks, :])
                        wih_sb.append(t_)
                    bias_sb = p1w.tile([128, G4], F32, tag=f"bias{d}")
                    nc.sync.dma_start(bias_sb[:], bias_d[d].ap())
                    for rc in range(RW):
                        ps = p1ps.tile([128, G4], F32, tag="pj")
                        for nj in range(4):
                            for ki in range(3):
                                nc.tensor.matmul(
                                    ps[:, nj * 512:(nj + 1) * 512],
                                    xT_sb[ki][:, rc * T:(rc + 1) * T],
                                    wih_sb[ki][:, nj * 512:(nj + 1) * 512],
                                    start=(ki == 0), stop=(ki == 2))
                        ev = p1e.tile([128, G4], F16, tag="ev")
                        nc.vector.tensor_add(ev[:], ps[:], bias_sb[:])
                        nc.sync.dma_start(xw[d][rc, :, :], ev[:])

            # ---------------- Phase 2: bidirectional LSTM scan ----------------
            if phases < 2:
                nc.compile()
                return nc
            with tc.tile_pool(name="wst", bufs=1) as wst, \
                 tc.tile_pool(name="sst", bufs=1) as sst, \
                 tc.tile_pool(name="xwp", bufs=xwp_bufs) as xwp, \
                 tc.tile_pool(name="gp", bufs=gp_bufs) as gp, \
                 tc.tile_pool(name="gps", bufs=1, space="PSUM") as gps_pool, \
                 tc.tile_pool(name="tps", bufs=2, space="PSUM") as tps_pool:
                whh_sb = {}
                for d in "fb":
                    whh_sb[d] = []
                    for kc in range(4):
                        w = wst.tile([128, G4], F32R, tag=f"whh{d}{kc}")
                        nc.sync.dma_start(w[:], whh_d[d].ap()[kc * 128:(kc + 1) * 128, :])
                        whh_sb[d].append(w)
                # hT_d: transposed h state per direction; chunk c in cols [32c:32c+32]
                hT = {d: sst.tile([128, 4 * RW], F32R, name=f"hT_{d}") for d in "fb"}
                c_st = {d: sst.tile([RW, H], F32, name=f"c_st_{d}") for d in "fb"}

                # gates layout (host permuted): [g | i | f | o]
                GG, GI, GF, GO = 0, 1, 2, 3
                for t in range(scan_T):
                    for di, d in enumerate("fb"):
                        tx = t if d == "f" else T - 1 - t
                        xwt = xwp.tile([RW, G4], F16, tag=f"xwt{d}", name=f"xwt{d}")
                        nc.sync.dma_start(xwt[:], xw[d][:, tx, :])
                        sgall = gp.tile([RW, G4], F32, tag=f"sgall{d}",
                                        name=f"sgall{d}")

                        def bank(nj):
                            return slice(nj * H, (nj + 1) * H)

                        if t == 0:
                            # h == 0: gates are just xw + bias (bias folded in xw)
                            for nj in range(4):
                                nc.vector.tensor_copy(sgall[:, bank(nj)],
                                                      xwt[:, bank(nj)])
                        else:
                            gps = gps_pool.tile([RW, G4], F32, tag=f"g{d}",
                                                name=f"gps{d}")
                            for nj in range(4):
                                for kc in range(4):
                                    nc.tensor.matmul(
                                        gps[:, bank(nj)],
                                        hT[d][:, 32 * kc:32 * kc + RW],
                                        whh_sb[d][kc][:, bank(nj)],
                                        start=(kc == 0), stop=(kc == 3))
                                nc.vector.tensor_add(sgall[:, bank(nj)],
                                                     gps[:, bank(nj)],
                                                     xwt[:, bank(nj)])
                        # activations in-place per gate; order [g, i, f, o]
                        nc.scalar.activation(sgall[:, bank(GG)], sgall[:, bank(GG)],
                                             AF.Tanh)
                        nc.scalar.activation(sgall[:, bank(GI)], sgall[:, bank(GI)],
                                             AF.Sigmoid)
                        p_ = gp.tile([RW, H], F32, tag=f"p_{d}", name=f"p_{d}")
                        nc.gpsimd.tensor_mul(p_[:], sgall[:, bank(GI)],
                                             sgall[:, bank(GG)])
                        nc.scalar.activation(sgall[:, bank(GF)], sgall[:, bank(GF)],
                                             AF.Sigmoid)
                        if t == 0:
                            nc.vector.tensor_copy(c_st[d][:], p_[:])
                        else:
                            q_ = gp.tile([RW, H], F32, tag=f"q_{d}", name=f"q_{d}")
                            nc.gpsimd.tensor_mul(q_[:], sgall[:, bank(GF)], c_st[d][:])
                            nc.vector.tensor_add(c_st[d][:], p_[:], q_[:])
                        nc.scalar.activation(sgall[:, bank(GO)], sgall[:, bank(GO)],
                                             AF.Sigmoid)
                        th = gp.tile([RW, H], F32, tag=f"th{d}", name=f"th{d}")
                        nc.scalar.activation(th[:], c_st[d][:], AF.Tanh)
                        h_ = gp.tile([RW, H], F32R, tag=f"h_{d}", name=f"h_{d}")
                        nc.vector.tensor_mul(h_[:], sgall[:, bank(GO)], th[:])
                        tp = gps_pool.tile([128, 4 * RW], F32R, tag=f"g{d}",
                                           name=f"tp{d}")
                        for cc in range(4):
                            nc.tensor.transpose(tp[:, RW * cc:RW * cc + RW],
                                                h_[:, 128 * cc:128 * cc + 128],
                                                identr[0:RW, 0:RW])
                        nc.vector.tensor_copy(hT[d][:], tp[:])
                        lo, hi = (0, H) if d == "f" else (H, H2)
                        nc.sync.dma_start(tm[:, tx, lo:hi], h_[:])
                        nc.sync.dma_start(outA_d.ap()[:, tx, lo:hi],
                                          h_[0:PB, :].bitcast(F32))
                        nc.sync.dma_start(outB_d.ap()[:, tx, lo:hi],
                                          h_[PB:RW, :].bitcast(F32))

            # ---------------- Phase 3: attention + enhancement ----------------
            if phases < 3:
                nc.compile()
                return nc
            with tc.tile_pool(name="a3", bufs=a3_bufs) as a3, \
                 tc.tile_pool(name="a3s", bufs=2) as a3s, \
                 tc.tile_pool(name="eps", bufs=eps_bufs, space="PSUM") as eps_pool, \
                 tc.tile_pool(name="tp3", bufs=3, space="PSUM") as tp3_pool, \
                 tc.tile_pool(name="ops", bufs=1, space="PSUM") as ops_pool:
                for n in range(PB):
                    a_tm = a3.tile([128, H2], F32R, tag="a_tm")
                    nc.sync.dma_start(a_tm[:], tm[n, :, :])
                    b_tm = a3.tile([128, H2], F32R, tag="b_tm")
                    nc.sync.dma_start(b_tm[:], tm[PB + n, :, :])
                    a_fm = a3.tile([128, H2], F32, tag="a_fm")
                    b_fm = a3.tile([128, H2], F32, tag="b_fm")
                    for src, dst in ((a_tm, a_fm), (b_tm, b_fm)):
                        for cc in range(8):
                            tp3 = tp3_pool.tile([128, 128], F32R, tag="tp3")
                            nc.tensor.transpose(tp3[:], src[:, 128 * cc:128 * (cc + 1)],
                                                identr[:])
                            nc.vector.tensor_copy(dst[:, 128 * cc:128 * (cc + 1)],
                                                  tp3[:].bitcast(F32))
                    e_ps = eps_pool.tile([128, 128], F32, tag="e")
                    e2_ps = eps_pool.tile([128, 128], F32, tag="e")
                    for cc in range(8):
                        sl = slice(128 * cc, 128 * (cc + 1))
                        nc.tensor.matmul(e_ps[:], a_fm[:, sl], b_fm[:, sl],
                                         start=(cc == 0), stop=(cc == 7))
                    for cc in range(8):
                        sl = slice(128 * cc, 128 * (cc + 1))
                        nc.tensor.matmul(e2_ps[:], b_fm[:, sl], a_fm[:, sl],
                                         start=(cc == 0), stop=(cc == 7))
                    zs, rs = [], []
                    for eps in (e_ps, e2_ps):
                        m_ = a3s.tile([128, 1], F32, tag="m_")
                        nc.vector.tensor_reduce(m_[:], eps[:], axis=AX.X,
                                                op=ALU.max, negate=True)
                        z_ = a3s.tile([128, 128], F32, tag="z_")
                        s_ = a3s.tile([128, 1], F32, tag="s_")
                        nc.scalar.activation(z_[:], eps[:], AF.Exp, bias=m_[:],
                                             accum_out=s_[:])
                        r_ = a3s.tile([128, 1], F32, tag="r_")
                        nc.vector.reciprocal(r_[:], s_[:])
                        zt_ps = tp3_pool.tile([128, 128], F32, tag="tp3")
                        nc.tensor.transpose(zt_ps[:], z_[:], ident[:])
                        zt = a3s.tile([128, 128], F32R, tag="zt")
                        nc.vector.tensor_copy(zt[:], zt_ps[:])
                        zs.append(zt)
                        rs.append(r_)
                    tilded = []
                    for zt, r_, rhs_tm in ((zs[0], rs[0], b_tm), (zs[1], rs[1], a_tm)):
                        t_ps = ops_pool.tile([128, H2], F32, tag="t_ps")
                        for half in range(2):
                            sl = slice(512 * half, 512 * (half + 1))
                            nc.tensor.matmul(t_ps[:, sl], zt[:], rhs_tm[:, sl],
                                             start=True, stop=True)
                        # assemble [til | diff | prod] contiguously, one DMA out
                        big = a3.tile([128, 3 * H2], F32, tag="big")
                        nc.vector.tensor_scalar_mul(big[:, 0:H2], t_ps[:], r_[:])
                        tilded.append(big)
                    for bar, big, outd in ((a_tm, tilded[0], outA_d),
                                           (b_tm, tilded[1], outB_d)):
                        nc.gpsimd.tensor_sub(big[:, H2:2 * H2], bar[:].bitcast(F32),
                                             big[:, 0:H2])
                        nc.vector.tensor_mul(big[:, 2 * H2:3 * H2],
                                             bar[:].bitcast(F32), big[:, 0:H2])
                        nc.sync.dma_start(outd.ap()[n, :, H2:4 * H2], big[:])

    nc.compile()
    return nc


def _get_nc():
    if "nc" not in _CACHE:
        _CACHE["nc"] = _build()
    return _CACHE["nc"]


def prep_in_maps(inputs):
    A = np.asarray(inputs["A"])
    B = np.asarray(inputs["B"])
    embed = np.asarray(inputs["embed"], dtype=np.float32)
    # permute pytorch gate order [i,f,g,o] -> [g,i,f,o]
    perm = np.concatenate([np.arange(2 * H, 3 * H), np.arange(0, 2 * H),
                           np.arange(3 * H, 4 * H)])
    wmat, bmat = {}, {}
    for d in "fb":
        suf = "_f" if d == "f" else "_b"
        wihT = np.ascontiguousarray(
            np.asarray(inputs["Wih" + suf], dtype=np.float32)[perm].T)
        whhT = np.ascontiguousarray(
            np.asarray(inputs["Whh" + suf], dtype=np.float32)[perm].T)
        bias = (np.asarray(inputs["bih" + suf], dtype=np.float32)
                + np.asarray(inputs["bhh" + suf], dtype=np.float32))[perm]
        bias_bc = np.ascontiguousarray(
            np.broadcast_to(bias[None, :], (128, G4)), dtype=np.float32)
        wmat[d] = (wihT, whhT)
        bmat[d] = bias_bc

    xa = embed[A]    # [BSZ, T, E]
    xb = embed[B]

    in_maps = []
    for c in range(NCORES):
        sl = slice(PB * c, PB * (c + 1))
        xc = np.concatenate([xa[sl], xb[sl]], axis=0)          # [RW, T, E]
        xT = np.ascontiguousarray(
            xc.transpose(2, 0, 1).reshape(E, RW * T), dtype=np.float32)
        in_maps.append({
            "xT": xT,
            "wihT_f": wmat["f"][0], "whhT_f": wmat["f"][1], "bias_f": bmat["f"],
            "wihT_b": wmat["b"][0], "whhT_b": wmat["b"][1], "bias_b": bmat["b"],
        })
    return in_maps


def kernel(**inputs):
    from concourse.bass_utils import run_bass_kernel_spmd

    in_maps = prep_in_maps(inputs)
    nc = _get_nc()
    res = run_bass_kernel_spmd(nc, in_maps, core_ids=list(range(NCORES)))
    outA = np.concatenate([res.results[c]["outA"] for c in range(NCORES)], axis=0)
    outB = np.concatenate([res.results[c]["outB"] for c in range(NCORES)], axis=0)
    return outA, outB


# ---------------------------------------------------------------------------
# Two-NEFF variant: run1 = proj + one (seq, dir, half-batch) scan per core;
# run2 = batch-sharded attention. Host reshuffles hidden states in between and
# writes the "bar" output quarter directly from run1's results.
B1 = 64  # batch rows per run1 core


def _build_run1():
    import concourse.mybir as mybir
    import concourse.tile as tile
    from concourse import bacc
    from concourse.masks import make_identity

    F32 = mybir.dt.float32
    F32R = mybir.dt.float32r
    F16 = mybir.dt.float16
    AF = mybir.ActivationFunctionType

    nc = bacc.Bacc("TRN2", target_bir_lowering=False, debug=False,
                   num_devices=NCORES)
    xT_d = nc.dram_tensor("xT", [E, B1 * T], F32R, kind="ExternalInput")
    wih_d = nc.dram_tensor("wihT", [E, G4], F32R, kind="ExternalInput")
    whh_d = nc.dram_tensor("whhT", [H, G4], F32R, kind="ExternalInput")
    bias_d = nc.dram_tensor("bias", [128, G4], F32, kind="ExternalInput")
    tm_d = nc.dram_tensor("tm1", [B1, T, H], F32, kind="ExternalOutput")

    with tile.TileContext(nc) as tc:
        with tc.tile_pool(name="dram", bufs=1, space="DRAM") as dpool, \
             tc.tile_pool(name="const", bufs=1) as const:
            xw = dpool.tile([B1, T, G4], F16, name="xw1")
            ident = const.tile([128, 128], F32)
            make_identity(nc, ident[:])
            identr = const.tile([128, 128], F32R)
            nc.vector.tensor_copy(identr[:], ident[:])

            # proj
            with tc.tile_pool(name="p1w", bufs=1) as p1w, \
                 tc.tile_pool(name="p1ps", bufs=2, space="PSUM") as p1ps, \
                 tc.tile_pool(name="p1e", bufs=3) as p1e:
                xT_sb, wih_sb = [], []
                for ki, (ko, ks) in enumerate(KCH):
                    t_ = p1w.tile([ks, B1 * T], F32R, tag=f"xT{ki}", name=f"xT{ki}")
                    nc.sync.dma_start(t_[:], xT_d.ap()[ko:ko + ks, :])
                    xT_sb.append(t_)
                    w_ = p1w.tile([ks, G4], F32R, tag=f"wih{ki}", name=f"wih{ki}")
                    nc.sync.dma_start(w_[:], wih_d.ap()[ko:ko + ks, :])
                    wih_sb.append(w_)
                bias_sb = p1w.tile([128, G4], F32, tag="bias")
                nc.sync.dma_start(bias_sb[:], bias_d.ap())
                for rc in range(B1):
                    ps = p1ps.tile([128, G4], F32, tag="pj")
                    for nj in range(4):
                        for ki in range(3):
                            nc.tensor.matmul(
                                ps[:, nj * 512:(nj + 1) * 512],
                                xT_sb[ki][:, rc * T:(rc + 1) * T],
                                wih_sb[ki][:, nj * 512:(nj + 1) * 512],
                                start=(ki == 0), stop=(ki == 2))
                    ev = p1e.tile([128, G4], F16, tag="ev")
                    nc.vector.tensor_add(ev[:], ps[:], bias_sb[:])
                    nc.sync.dma_start(xw[rc, :, :], ev[:])

            # scan (single direction; bwd cores get host-reversed inputs)
            with tc.tile_pool(name="wst", bufs=1) as wst, \
                 tc.tile_pool(name="sst", bufs=1) as sst, \
                 tc.tile_pool(name="xwp", bufs=3) as xwp, \
                 tc.tile_pool(name="gp", bufs=2) as gp, \
                 tc.tile_pool(name="gps", bufs=1, space="PSUM") as gps_pool:
                whh_sb = []
                for kc in range(4):
                    w = wst.tile([128, G4], F32R, tag=f"whh{kc}", name=f"whh{kc}")
                    nc.sync.dma_start(w[:], whh_d.ap()[kc * 128:(kc + 1) * 128, :])
                    whh_sb.append(w)
                hT = sst.tile([128, 4 * B1], F32R, name="hT1")
                c_st = sst.tile([B1, H], F32, name="c_st1")

                GG, GI, GF, GO = 0, 1, 2, 3
                for t in range(T):
                    xwt = xwp.tile([B1, G4], F16, tag="xwt", name="xwt")
                    nc.sync.dma_start(xwt[:], xw[:, t, :])
                    sgall = gp.tile([B1, G4], F32, tag="sgall", name="sgall")

                    def bank(nj):
                        return slice(nj * H, (nj + 1) * H)

                    if t == 0:
                        for nj in range(4):
                            nc.vector.tensor_copy(sgall[:, bank(nj)],
                                                  xwt[:, bank(nj)])
                    else:
                        gps = gps_pool.tile([B1, G4], F32, tag="g", name="gps1")
                        for nj in range(4):
                            for kc in range(4):
                                nc.tensor.matmul(
                                    gps[:, bank(nj)],
                                    hT[:, B1 * kc:B1 * kc + B1],
                                    whh_sb[kc][:, bank(nj)],
                                    start=(kc == 0), stop=(kc == 3))
                            nc.vector.tensor_add(sgall[:, bank(nj)],
                                                 gps[:, bank(nj)],
                                                 xwt[:, bank(nj)])
                    nc.scalar.activation(sgall[:, bank(GG)], sgall[:, bank(GG)],
                                         AF.Tanh)
                    nc.scalar.activation(sgall[:, bank(GI)], sgall[:, bank(GI)],
                                         AF.Sigmoid)
                    p_ = gp.tile([B1, H], F32, tag="p_", name="p_")
                    nc.gpsimd.tensor_mul(p_[:], sgall[:, bank(GI)],
                                         sgall[:, bank(GG)])
                    nc.scalar.activation(sgall[:, bank(GF)], sgall[:, bank(GF)],
                                         AF.Sigmoid)
                    if t == 0:
                        nc.vector.tensor_copy(c_st[:], p_[:])
                    else:
                        q_ = gp.tile([B1, H], F32, tag="q_", name="q_")
                        nc.gpsimd.tensor_mul(q_[:], sgall[:, bank(GF)], c_st[:])
                        nc.vector.tensor_add(c_st[:], p_[:], q_[:])
                    nc.scalar.activation(sgall[:, bank(GO)], sgall[:, bank(GO)],
                                         AF.Sigmoid)
                    th = gp.tile([B1, H], F32, tag="th", name="th")
                    nc.scalar.activation(th[:], c_st[:], AF.Tanh)
                    h_ = gp.tile([B1, H], F32, tag="h_", name="h_")
                    nc.vector.tensor_mul(h_[:], sgall[:, bank(GO)], th[:])
                    tp = gps_pool.tile([128, 4 * B1], F32R, tag="g", name="tp1")
                    for cc in range(4):
                        nc.tensor.transpose(tp[:, B1 * cc:B1 * cc + B1],
                                            h_[:, 128 * cc:128 * cc + 128]
                                            .bitcast(F32R),
                                            identr[0:B1, 0:B1])
                    nc.vector.tensor_copy(hT[:], tp[:])
                    nc.sync.dma_start(tm_d.ap()[:, t, :], h_[:])
    nc.compile()
    return nc


def _build_run2():
    import concourse.mybir as mybir
    import concourse.tile as tile
    from concourse import bacc
    from concourse.masks import make_identity

    F32 = mybir.dt.float32
    F32R = mybir.dt.float32r
    AF = mybir.ActivationFunctionType
    ALU = mybir.AluOpType
    AX = mybir.AxisListType

    nc = bacc.Bacc("TRN2", target_bir_lowering=False, debug=False,
                   num_devices=NCORES)
    tmA_d = nc.dram_tensor("tmA", [PB, T, H2], F32R, kind="ExternalInput")
    tmB_d = nc.dram_tensor("tmB", [PB, T, H2], F32R, kind="ExternalInput")
    oA_d = nc.dram_tensor("oA", [PB, T, 3 * H2], F32, kind="ExternalOutput")
    oB_d = nc.dram_tensor("oB", [PB, T, 3 * H2], F32, kind="ExternalOutput")

    with tile.TileContext(nc) as tc:
        with tc.tile_pool(name="const", bufs=1) as const, \
             tc.tile_pool(name="a3", bufs=2) as a3, \
             tc.tile_pool(name="a3s", bufs=2) as a3s, \
             tc.tile_pool(name="eps", bufs=2, space="PSUM") as eps_pool, \
             tc.tile_pool(name="tp3", bufs=3, space="PSUM") as tp3_pool, \
             tc.tile_pool(name="ops", bufs=1, space="PSUM") as ops_pool:
            ident = const.tile([128, 128], F32)
            make_identity(nc, ident[:])
            identr = const.tile([128, 128], F32R)
            nc.vector.tensor_copy(identr[:], ident[:])
            for n in range(PB):
                a_tm = a3.tile([128, H2], F32R, tag="a_tm")
                nc.sync.dma_start(a_tm[:], tmA_d.ap()[n, :, :])
                b_tm = a3.tile([128, H2], F32R, tag="b_tm")
                nc.sync.dma_start(b_tm[:], tmB_d.ap()[n, :, :])
                a_fm = a3.tile([128, H2], F32, tag="a_fm")
                b_fm = a3.tile([128, H2], F32, tag="b_fm")
                for src_, dst in ((a_tm, a_fm), (b_tm, b_fm)):
                    for cc in range(8):
                        tp3 = tp3_pool.tile([128, 128], F32R, tag="tp3")
                        nc.tensor.transpose(tp3[:], src_[:, 128 * cc:128 * (cc + 1)],
                                            identr[:])
                        nc.vector.tensor_copy(dst[:, 128 * cc:128 * (cc + 1)],
                                              tp3[:].bitcast(F32))
                e_ps = eps_pool.tile([128, 128], F32, tag="e")
                e2_ps = eps_pool.tile([128, 128], F32, tag="e")
                for cc in range(8):
                    sl = slice(128 * cc, 128 * (cc + 1))
                    nc.tensor.matmul(e_ps[:], a_fm[:, sl], b_fm[:, sl],
                                     start=(cc == 0), stop=(cc == 7))
                for cc in range(8):
                    sl = slice(128 * cc, 128 * (cc + 1))
                    nc.tensor.matmul(e2_ps[:], b_fm[:, sl], a_fm[:, sl],
                                     start=(cc == 0), stop=(cc == 7))
                zs, rs = [], []
                for eps in (e_ps, e2_ps):
                    m_ = a3s.tile([128, 1], F32, tag="m_")
                    nc.vector.tensor_reduce(m_[:], eps[:], axis=AX.X,
                                            op=ALU.max, negate=True)
                    z_ = a3s.tile([128, 128], F32, tag="z_")
                    s_ = a3s.tile([128, 1], F32, tag="s_")
                    nc.scalar.activation(z_[:], eps[:], AF.Exp, bias=m_[:],
                                         accum_out=s_[:])
                    r_ = a3s.tile([128, 1], F32, tag="r_")
                    nc.vector.reciprocal(r_[:], s_[:])
                    zt_ps = tp3_pool.tile([128, 128], F32, tag="tp3")
                    nc.tensor.transpose(zt_ps[:], z_[:], ident[:])
                    zt = a3s.tile([128, 128], F32R, tag="zt")
                    nc.vector.tensor_copy(zt[:], zt_ps[:])
                    zs.append(zt)
                    rs.append(r_)
                tilded = []
                for zt, r_, rhs_tm in ((zs[0], rs[0], b_tm), (zs[1], rs[1], a_tm)):
                    t_ps = ops_pool.tile([128, H2], F32, tag="t_ps")
                    for half in range(2):
                        sl = slice(512 * half, 512 * (half + 1))
                        nc.tensor.matmul(t_ps[:, sl], zt[:], rhs_tm[:, sl],
                                         start=True, stop=True)
                    til = a3.tile([128, H2], F32, tag="til")
                    nc.vector.tensor_scalar_mul(til[:], t_ps[:], r_[:])
                    tilded.append(til)
                for bar, til, outd in ((a_tm, tilded[0], oA_d),
                                       (b_tm, tilded[1], oB_d)):
                    nc.sync.dma_start(outd.ap()[n, :, 0:H2], til[:])
                    df = a3.tile([128, H2], F32, tag="df")
                    nc.gpsimd.tensor_sub(df[:], bar[:].bitcast(F32), til[:])
                    nc.sync.dma_start(outd.ap()[n, :, H2:2 * H2], df[:])
                    pr = a3.tile([128, H2], F32, tag="pr")
                    nc.vector.tensor_mul(pr[:], bar[:].bitcast(F32), til[:])
                    nc.sync.dma_start(outd.ap()[n, :, 2 * H2:3 * H2], pr[:])
    nc.compile()
    return nc


def kernel2(**inputs):
    """Two-NEFF variant: run1 scans, host reshuffle, run2 attention."""
    from concourse.bass_utils import run_bass_kernel_spmd

    A = np.asarray(inputs["A"])
    B = np.asarray(inputs["B"])
    embed = np.asarray(inputs["embed"], dtype=np.float32)
    perm = np.concatenate([np.arange(2 * H, 3 * H), np.arange(0, 2 * H),
                           np.arange(3 * H, 4 * H)])
    wp = {}
    for d in "fb":
        suf = "_f" if d == "f" else "_b"
        wihT = np.ascontiguousarray(
            np.asarray(inputs["Wih" + suf], dtype=np.float32)[perm].T)
        whhT = np.ascontiguousarray(
            np.asarray(inputs["Whh" + suf], dtype=np.float32)[perm].T)
        bias = (np.asarray(inputs["bih" + suf], dtype=np.float32)
                + np.asarray(inputs["bhh" + suf], dtype=np.float32))[perm]
        bias_bc = np.ascontiguousarray(
            np.broadcast_to(bias[None, :], (128, G4)), dtype=np.float32)
        wp[d] = (wihT, whhT, bias_bc)

    x_seq = {0: embed[A], 1: embed[B]}   # [BSZ, T, E]

    # core c: seq = c//4, dir = (c//2)%2 (0=f,1=b), half = c%2
    in_maps1 = []
    meta = []
    for c in range(NCORES):
        seq, dirb, half = c // 4, (c // 2) % 2, c % 2
        d = "fb"[dirb]
        xs = x_seq[seq][B1 * half:B1 * (half + 1)]       # [64, T, E]
        if d == "b":
            xs = xs[:, ::-1, :]                          # reversed time
        xT = np.ascontiguousarray(
            xs.transpose(2, 0, 1).reshape(E, B1 * T), dtype=np.float32)
        wihT, whhT, bias_bc = wp[d]
        in_maps1.append({"xT": xT, "wihT": wihT, "whhT": whhT, "bias": bias_bc})
        meta.append((seq, d, half))

    if "nc1" not in _CACHE:
        _CACHE["nc1"] = _build_run1()
    res1 = run_bass_kernel_spmd(_CACHE["nc1"], in_maps1,
                                core_ids=list(range(NCORES)))

    tm_full = {0: np.empty((BSZ, T, H2), np.float32),
               1: np.empty((BSZ, T, H2), np.float32)}
    for c, (seq, d, half) in enumerate(meta):
        tm1 = res1.results[c]["tm1"]                     # [64, T, 512]
        if d == "b":
            tm1 = tm1[:, ::-1, :]
        lo = 0 if d == "f" else H
        tm_full[seq][B1 * half:B1 * (half + 1), :, lo:lo + H] = tm1

    in_maps2 = []
    for c in range(NCORES):
        sl = slice(PB * c, PB * (c + 1))
        in_maps2.append({
            "tmA": np.ascontiguousarray(tm_full[0][sl]),
            "tmB": np.ascontiguousarray(tm_full[1][sl]),
        })
    if "nc2" not in _CACHE:
        _CACHE["nc2"] = _build_run2()
    res2 = run_bass_kernel_spmd(_CACHE["nc2"], in_maps2,
                                core_ids=list(range(NCORES)))

    outA = np.empty((BSZ, T, 4 * H2), np.float32)
    outB = np.empty((BSZ, T, 4 * H2), np.float32)
    outA[:, :, 0:H2] = tm_full[0]
    outB[:, :, 0:H2] = tm_full[1]
    for c in range(NCORES):
        sl = slice(PB * c, PB * (c + 1))
        outA[sl, :, H2:] = res2.results[c]["oA"]
        outB[sl, :, H2:] = res2.results[c]["oB"]
    return outA, outB



# revision 15
# speedup vs baseline: 2.6869x; 2.6869x over previous
"""Bass/Trainium2 kernel for nn_Encoder (embedding -> BiLSTM -> cross attention
-> enhancement).

Sharding: data-parallel over batch, 16 items per core on 8 NeuronCores (no
collectives). Per core the A and B sequences are stacked into RW=32 rows.

The BiLSTM scan runs FEATURE-MAJOR: all state (gates, c, h) lives in
[128-feature-partition x 32-row] tiles, so the recurrence needs no PE
transposes and every elementwise op uses all 128 partitions. The input
projection x@Wih^T + bias is fused into the gate PSUM accumulation as extra
matmuls against a bias-augmented [301, 2048] weight (xT carries a constant
ones row), issued one step ahead so they fill PE gaps while the current
step's elementwise chain runs. h is written once per step as f16 directly
into a persistent SBUF buffer that both the next step's matmuls and the
attention phase consume.

Attention: per item, PE transposes build time-major [T, H2] "bar" tiles from
the SBUF h buffer, E/E^T come from feature-major matmuls, softmax via
Exp-with-accum, soft alignment matmuls, and the 4-way enhancement concat is
assembled in one [128, 4096] f16 tile and written with a single DMA per
sequence. Outputs are f16 in DRAM; the host converts to f32.
"""

import numpy as np

V, E, H = 32000, 300, 512
BSZ, T = 128, 128
NCORES = 8
PB = BSZ // NCORES          # 16 batch items per core
RW = 2 * PB                 # 32 stacked rows (A items then B items)
G4 = 4 * H                  # 2048 gate width
H2 = 2 * H                  # 1024 bilstm output width
EB = E + 1                  # embedding dim + folded bias row
KCH = [(0, 128), (128, 128), (256, EB - 256)]   # chunks of EB=301
OUT = 4 * H2

_CACHE = {}


def _build():
    import concourse.mybir as mybir
    import concourse.tile as tile
    from concourse import bacc
    from concourse.masks import make_identity

    F32 = mybir.dt.float32
    F16 = mybir.dt.float16
    AF = mybir.ActivationFunctionType
    ALU = mybir.AluOpType
    AX = mybir.AxisListType

    nc = bacc.Bacc("TRN2", target_bir_lowering=False, debug=False,
                   num_devices=NCORES)

    xT_d = nc.dram_tensor("xTb", [EB, T, RW], F16, kind="ExternalInput")
    wih_d = {d: nc.dram_tensor(f"wihT_{d}", [EB, G4], F16, kind="ExternalInput")
             for d in "fb"}
    whh_d = {d: nc.dram_tensor(f"whhT_{d}", [H, G4], F16, kind="ExternalInput")
             for d in "fb"}
    outA_d = nc.dram_tensor("outA", [PB, T, OUT], F16, kind="ExternalOutput")
    outB_d = nc.dram_tensor("outB", [PB, T, OUT], F16, kind="ExternalOutput")

    # gate column layout in psum: [g | i | f | o], each 128 wide (4 fc of 32)
    SG = slice(0, 128)
    SI = slice(128, 256)
    SF = slice(256, 384)
    SO = slice(384, 512)

    with tile.TileContext(nc) as tc:
        with tc.tile_pool(name="const", bufs=1) as const, \
             tc.tile_pool(name="hbuf", bufs=1) as hbuf:
            identf = const.tile([128, 128], F32)
            make_identity(nc, identf[:])
            ident16 = const.tile([128, 128], F16)
            nc.vector.tensor_copy(ident16[:], identf[:])
            # persistent h buffers: [feature-in-chunk, t, kc, row], f16
            Hb = {d: hbuf.tile([128, T, 4, RW], F16, name=f"Hb_{d}")
                  for d in "fb"}

            # ---------------- Phase 1+2: fused proj + scan ----------------
            with tc.tile_pool(name="wst", bufs=1) as wst, \
                 tc.tile_pool(name="sst", bufs=1) as sst, \
                 tc.tile_pool(name="ew", bufs=2) as ew, \
                 tc.tile_pool(name="gps", bufs=3, space="PSUM") as gps_pool:
                xT_sb = []
                qs = [nc.sync, nc.scalar, nc.gpsimd]
                qi = 0
                for ki, (ko, ks) in enumerate(KCH):
                    t_ = wst.tile([ks, T, RW], F16, tag=f"xT{ki}")
                    qs[qi % 3].dma_start(t_[:], xT_d.ap()[ko:ko + ks])
                    qi += 1
                    xT_sb.append(t_)
                wih_sb, whh_sb = {}, {}
                for d in "fb":
                    wih_sb[d] = []
                    for ki, (ko, ks) in enumerate(KCH):
                        w = wst.tile([ks, G4], F16, tag=f"wih{d}{ki}")
                        qs[qi % 3].dma_start(w[:], wih_d[d].ap()[ko:ko + ks])
                        qi += 1
                        wih_sb[d].append(w)
                    whh_sb[d] = []
                    for kr in range(4):
                        w = wst.tile([128, G4], F16, tag=f"whh{d}{kr}")
                        qs[qi % 3].dma_start(
                            w[:], whh_d[d].ap()[kr * 128:(kr + 1) * 128])
                        qi += 1
                        whh_sb[d].append(w)
                # c state in f16: every cell-update DVE op is then a packed
                # 2-byte all-SBUF op (half-rate cycles on DVE)
                c_st = {d: sst.tile([128, 128], F16, name=f"c_{d}")
                        for d in "fb"}

                def x_accum(t, d):
                    """Issue x-part (incl bias) matmuls for step t into a
                    fresh psum tile. One accumulation group per bank-tile:
                    start=True only on the very first matmul (it clears
                    has_written for the whole bank), stop=True only on the
                    tile's last matmul (at t==0 that is the last x matmul,
                    otherwise the last Whh matmul issued later)."""
                    tx = t if d == "f" else T - 1 - t
                    g = gps_pool.tile([128, 16 * RW], F32, tag=f"g{d}",
                                      name=f"g{d}")
                    for fc in range(16):
                        fs = slice(fc * 128, (fc + 1) * 128)
                        for ki in range(3):
                            nc.tensor.matmul(
                                g[:, fc * RW:(fc + 1) * RW],
                                wih_sb[d][ki][:, fs],
                                xT_sb[ki][:, tx, :],
                                start=(fc == 0 and ki == 0),
                                stop=(t == 0 and fc == 15 and ki == 2))
                    return g

                gcur = {d: x_accum(0, d) for d in "fb"}

                for t in range(T):
                    for d in "fb":
                        tx = t if d == "f" else T - 1 - t
                        txp = tx - 1 if d == "f" else tx + 1
                        g = gcur[d]
                        if t > 0:
                            for fc in range(16):
                                fs = slice(fc * 128, (fc + 1) * 128)
                                for kr in range(4):
                                    nc.tensor.matmul(
                                        g[:, fc * RW:(fc + 1) * RW],
                                        whh_sb[d][kr][:, fs],
                                        Hb[d][:, txp, kr, :],
                                        start=False,
                                        stop=(fc == 15 and kr == 3))
                        # elementwise, all [128, 128] feature-major.
                        # tanh(g) lands in SBUF st[:,0] (DVE may read only
                        # one PSUM operand); c state lives in st[:,1].
                        sd = st[d]
                        nc.scalar.activation(sd[:, 0, :], g[:, SG], AF.Tanh)
                        nc.scalar.activation(g[:, SI.start:SF.stop],
                                             g[:, SI.start:SF.stop],
                                             AF.Sigmoid)
                        if t == 0:
                            nc.vector.tensor_mul(sd[:, 1, :], g[:, SI],
                                                 sd[:, 0, :])
                        else:
                            # pq[:, c, 0] = sig(i)*tanh(g), pq[:, c, 1] =
                            # sig(f)*c ; then c = pq.sum(axis=-1)
                            pq = ew.tile([128, 128, 2], F32, tag=f"pq{d}")
                            nc.vector.tensor_mul(
                                pq[:, :, :],
                                g[:, SI.start:SF.stop].rearrange(
                                    "p (j c) -> p c j", j=2),
                                sd[:, :, :].rearrange("p j c -> p c j"))
                            nc.vector.tensor_reduce(sd[:, 1, :], pq[:, :, :],
                                                    axis=AX.X, op=ALU.add)
                        nc.scalar.activation(g[:, SO], g[:, SO], AF.Sigmoid)
                        tc_ = ew.tile([128, 128], F32, tag=f"tc{d}")
                        nc.scalar.activation(tc_[:], sd[:, 1, :], AF.Tanh)
                        nc.vector.tensor_mul(Hb[d][:, tx, :, :], g[:, SO],
                                             tc_[:])
                        if t + 1 < T:
                            gcur[d] = x_accum(t + 1, d)

            # ---------------- Phase 3: attention + enhancement ----------------
            with tc.tile_pool(name="abuf", bufs=2) as abuf, \
                 tc.tile_pool(name="zbuf", bufs=2) as zbuf, \
                 tc.tile_pool(name="tmps", bufs=1, space="PSUM") as tmps, \
                 tc.tile_pool(name="eps", bufs=1, space="PSUM") as eps_pool, \
                 tc.tile_pool(name="tilps", bufs=1, space="PSUM") as til_pool:
                for n in range(PB):
                    bigs = {}
                    for s, row in (("a", n), ("b", PB + n)):
                        big = abuf.tile([128, OUT], F16, tag=f"big{s}")
                        tm_ps = tmps.tile([128, H2], F16, tag="tm")
                        for c8 in range(8):
                            d, kc = "fb"[c8 // 4], c8 % 4
                            nc.tensor.transpose(
                                tm_ps[:, c8 * 128:(c8 + 1) * 128],
                                Hb[d][:, :, kc, row], ident16[:])
                        nc.vector.tensor_copy(big[:, 0:H2], tm_ps[:])
                        bigs[s] = big
                    e_ps = eps_pool.tile([128, 128], F32, tag="e1")
                    e2_ps = eps_pool.tile([128, 128], F32, tag="e2")
                    for c8 in range(8):
                        d, kc = "fb"[c8 // 4], c8 % 4
                        asl = Hb[d][:, :, kc, n]
                        bsl = Hb[d][:, :, kc, PB + n]
                        nc.tensor.matmul(e_ps[:], asl, bsl,
                                         start=(c8 == 0), stop=(c8 == 7))
                    for c8 in range(8):
                        d, kc = "fb"[c8 // 4], c8 % 4
                        asl = Hb[d][:, :, kc, n]
                        bsl = Hb[d][:, :, kc, PB + n]
                        nc.tensor.matmul(e2_ps[:], bsl, asl,
                                         start=(c8 == 0), stop=(c8 == 7))
                    for ei, (ep, rhs_s, dst_s) in enumerate(
                            ((e_ps, "b", "a"), (e2_ps, "a", "b"))):
                        m_ = zbuf.tile([128, 1], F32, tag=f"m{ei}")
                        nc.vector.tensor_reduce(m_[:], ep[:], axis=AX.X,
                                                op=ALU.max, negate=True)
                        z_ = zbuf.tile([128, 128], F16, tag=f"z{ei}")
                        s_ = zbuf.tile([128, 1], F32, tag=f"s{ei}")
                        nc.scalar.activation(z_[:], ep[:], AF.Exp, bias=m_[:],
                                             accum_out=s_[:])
                        r_ = zbuf.tile([128, 1], F32, tag=f"r{ei}")
                        nc.vector.reciprocal(r_[:], s_[:])
                        ztp = eps_pool.tile([128, 128], F16, tag="ztp")
                        nc.tensor.transpose(ztp[:], z_[:], ident16[:])
                        zt = zbuf.tile([128, 128], F16, tag=f"zt{ei}")
                        nc.vector.tensor_copy(zt[:], ztp[:])
                        til_ps = til_pool.tile([128, H2], F32, tag=f"til{ei}")
                        rhs = bigs[rhs_s]
                        for hh in range(2):
                            sl = slice(512 * hh, 512 * (hh + 1))
                            nc.tensor.matmul(til_ps[:, sl], zt[:], rhs[:, sl],
                                             start=True, stop=True)
                        dst = bigs[dst_s]
                        nc.scalar.activation(dst[:, H2:2 * H2], til_ps[:],
                                             AF.Copy, scale=r_[:])
                        # diff/prod: all-SBUF f16, split halves DVE/Pool
                        nc.vector.tensor_sub(dst[:, 2 * H2:2 * H2 + 512],
                                             dst[:, 0:512], dst[:, H2:H2 + 512])
                        nc.gpsimd.tensor_sub(dst[:, 2 * H2 + 512:3 * H2],
                                             dst[:, 512:H2],
                                             dst[:, H2 + 512:2 * H2])
                        nc.gpsimd.tensor_mul(dst[:, 3 * H2:3 * H2 + 512],
                                             dst[:, 0:512], dst[:, H2:H2 + 512])
                        nc.vector.tensor_mul(dst[:, 3 * H2 + 512:4 * H2],
                                             dst[:, 512:H2],
                                             dst[:, H2 + 512:2 * H2])
                    nc.sync.dma_start(outA_d.ap()[n], bigs["a"][:])
                    nc.sync.dma_start(outB_d.ap()[n], bigs["b"][:])

    nc.compile()
    return nc


def _get_nc():
    if "nc" not in _CACHE:
        _CACHE["nc"] = _build()
    return _CACHE["nc"]


def prep_in_maps(inputs):
    A = np.asarray(inputs["A"])
    B = np.asarray(inputs["B"])
    embed = np.asarray(inputs["embed"], dtype=np.float32)
    # permute pytorch gate order [i,f,g,o] -> [g,i,f,o]
    perm = np.concatenate([np.arange(2 * H, 3 * H), np.arange(0, 2 * H),
                           np.arange(3 * H, 4 * H)])
    wmat = {}
    for d in "fb":
        suf = "_" + d
        wihT = np.asarray(inputs["Wih" + suf], dtype=np.float32)[perm].T
        bias = (np.asarray(inputs["bih" + suf], dtype=np.float32)
                + np.asarray(inputs["bhh" + suf], dtype=np.float32))[perm]
        wihT_aug = np.concatenate([wihT, bias[None, :]], axis=0)  # [301, 2048]
        whhT = np.asarray(inputs["Whh" + suf], dtype=np.float32)[perm].T
        wmat[d] = (np.ascontiguousarray(wihT_aug, dtype=np.float16),
                   np.ascontiguousarray(whhT, dtype=np.float16))

    xa = embed[A]    # [BSZ, T, E]
    xb = embed[B]

    in_maps = []
    for c in range(NCORES):
        sl = slice(PB * c, PB * (c + 1))
        xc = np.concatenate([xa[sl], xb[sl]], axis=0)          # [RW, T, E]
        xT = xc.transpose(2, 1, 0)                             # [E, T, RW]
        xTb = np.concatenate(
            [xT, np.ones((1, T, RW), np.float32)], axis=0)     # [EB, T, RW]
        in_maps.append({
            "xTb": np.ascontiguousarray(xTb, dtype=np.float16),
            "wihT_f": wmat["f"][0], "whhT_f": wmat["f"][1],
            "wihT_b": wmat["b"][0], "whhT_b": wmat["b"][1],
        })
    return in_maps


def kernel(**inputs):
    from concourse.bass_utils import run_bass_kernel_spmd

    in_maps = prep_in_maps(inputs)
    nc = _get_nc()
    res = run_bass_kernel_spmd(nc, in_maps, core_ids=list(range(NCORES)))
    outA = np.concatenate(
        [res.results[c]["outA"] for c in range(NCORES)], axis=0)
    outB = np.concatenate(
        [res.results[c]["outB"] for c in range(NCORES)], axis=0)
    return outA.astype(np.float32), outB.astype(np.float32)


# revision 38
# speedup vs baseline: 3.0479x; 1.1344x over previous
"""Bass/Trainium2 kernel for nn_Encoder (embedding -> BiLSTM -> cross attention
-> enhancement).

Sharding: data-parallel over batch, 16 items per core on 8 NeuronCores (no
collectives). Per core the A and B sequences are stacked into RW=32 rows.

The BiLSTM scan runs FEATURE-MAJOR: all state (gates, c, h) lives in
[128-feature-partition x 32-row] tiles, so the recurrence needs no PE
transposes and every elementwise op uses all 128 partitions. The input
projection x@Wih^T + bias is fused into the gate PSUM accumulation as extra
matmuls against a bias-augmented [301, 2048] weight (xT carries a constant
ones row), issued one step ahead so they fill PE gaps while the current
step's elementwise chain runs. h is written once per step as f16 directly
into a persistent SBUF buffer that both the next step's matmuls and the
attention phase consume.

Attention: per item, PE transposes build time-major [T, H2] "bar" tiles from
the SBUF h buffer, E/E^T come from feature-major matmuls, softmax via
Exp-with-accum, soft alignment matmuls, and the 4-way enhancement concat is
assembled in one [128, 4096] f16 tile and written with a single DMA per
sequence. Outputs are f16 in DRAM; the host converts to f32.
"""

import numpy as np

V, E, H = 32000, 300, 512
BSZ, T = 128, 128
NCORES = 8
PB = BSZ // NCORES          # 16 batch items per core
RW = 2 * PB                 # 32 stacked rows (A items then B items)
G4 = 4 * H                  # 2048 gate width
H2 = 2 * H                  # 1024 bilstm output width
EB = E + 1                  # embedding dim + folded bias row
KCH = [(0, 128), (128, 128), (256, EB - 256)]   # chunks of EB=301
OUT = 4 * H2

_CACHE = {}


def _build():
    import concourse.mybir as mybir
    import concourse.tile as tile
    from concourse import bacc
    from concourse.masks import make_identity

    F32 = mybir.dt.float32
    F16 = mybir.dt.float16
    AF = mybir.ActivationFunctionType
    ALU = mybir.AluOpType
    AX = mybir.AxisListType

    nc = bacc.Bacc("TRN2", target_bir_lowering=False, debug=False,
                   num_devices=NCORES)

    xT_d = nc.dram_tensor("xTb", [EB, T, RW], F16, kind="ExternalInput")
    wih_d = {d: nc.dram_tensor(f"wihT_{d}", [EB, G4], F16, kind="ExternalInput")
             for d in "fb"}
    whh_d = {d: nc.dram_tensor(f"whhT_{d}", [H, G4], F16, kind="ExternalInput")
             for d in "fb"}
    outA_d = nc.dram_tensor("outA", [PB, T, OUT], F16, kind="ExternalOutput")
    outB_d = nc.dram_tensor("outB", [PB, T, OUT], F16, kind="ExternalOutput")

    # gate column layout in psum: [g | i | f | o], each 128 wide (4 fc of 32)
    SG = slice(0, 128)
    SI = slice(128, 256)
    SF = slice(256, 384)
    SO = slice(384, 512)

    with tile.TileContext(nc) as tc:
        with tc.tile_pool(name="const", bufs=1) as const, \
             tc.tile_pool(name="hbuf", bufs=1) as hbuf:
            identf = const.tile([128, 128], F32)
            make_identity(nc, identf[:])
            ident16 = const.tile([128, 128], F16)
            nc.vector.tensor_copy(ident16[:], identf[:])
            # persistent h buffers: [feature-in-chunk, t, kc, row], f16
            Hb = {d: hbuf.tile([128, T, 4, RW], F16, name=f"Hb_{d}")
                  for d in "fb"}

            # ---------------- Phase 1+2: fused proj + scan ----------------
            with tc.tile_pool(name="wst", bufs=1) as wst, \
                 tc.tile_pool(name="sst", bufs=1) as sst, \
                 tc.tile_pool(name="ew", bufs=2) as ew, \
                 tc.tile_pool(name="gps", bufs=3, space="PSUM") as gps_pool:
                xT_sb = []
                qs = [nc.sync, nc.scalar, nc.gpsimd]
                qi = 0
                for ki, (ko, ks) in enumerate(KCH):
                    t_ = wst.tile([ks, T, RW], F16, tag=f"xT{ki}")
                    qs[qi % 3].dma_start(t_[:], xT_d.ap()[ko:ko + ks])
                    qi += 1
                    xT_sb.append(t_)
                wih_sb, whh_sb = {}, {}
                for d in "fb":
                    wih_sb[d] = []
                    for ki, (ko, ks) in enumerate(KCH):
                        w = wst.tile([ks, G4], F16, tag=f"wih{d}{ki}")
                        qs[qi % 3].dma_start(w[:], wih_d[d].ap()[ko:ko + ks])
                        qi += 1
                        wih_sb[d].append(w)
                    whh_sb[d] = []
                    for kr in range(4):
                        w = wst.tile([128, G4], F16, tag=f"whh{d}{kr}")
                        qs[qi % 3].dma_start(
                            w[:], whh_d[d].ap()[kr * 128:(kr + 1) * 128])
                        qi += 1
                        whh_sb[d].append(w)
                # c state in f16: every cell-update DVE op is then a packed
                # 2-byte all-SBUF op (half-rate cycles on DVE)
                c_st = {d: sst.tile([128, 128], F16, name=f"c_{d}")
                        for d in "fb"}

                def x_accum(t, d):
                    """Issue x-part (incl bias) matmuls for step t into a
                    fresh psum tile. One accumulation group per bank-tile:
                    start=True only on the very first matmul (it clears
                    has_written for the whole bank), stop=True only on the
                    tile's last matmul (at t==0 that is the last x matmul,
                    otherwise the last Whh matmul issued later)."""
                    tx = t if d == "f" else T - 1 - t
                    g = gps_pool.tile([128, 16 * RW], F32, tag=f"g{d}",
                                      name=f"g{d}")
                    for fc in range(16):
                        fs = slice(fc * 128, (fc + 1) * 128)
                        for ki in range(3):
                            nc.tensor.matmul(
                                g[:, fc * RW:(fc + 1) * RW],
                                wih_sb[d][ki][:, fs],
                                xT_sb[ki][:, tx, :],
                                start=(fc == 0 and ki == 0),
                                stop=(t == 0 and fc == 15 and ki == 2))
                    return g

                gcur = {d: x_accum(0, d) for d in "fb"}

                for t in range(T):
                    for d in "fb":
                        tx = t if d == "f" else T - 1 - t
                        txp = tx - 1 if d == "f" else tx + 1
                        g = gcur[d]
                        if t > 0:
                            for fc in range(16):
                                fs = slice(fc * 128, (fc + 1) * 128)
                                for kr in range(4):
                                    nc.tensor.matmul(
                                        g[:, fc * RW:(fc + 1) * RW],
                                        whh_sb[d][kr][:, fs],
                                        Hb[d][:, txp, kr, :],
                                        start=False,
                                        stop=(fc == 15 and kr == 3))
                        # elementwise, all [128, 128] feature-major. ACT
                        # moves activated gates PSUM -> packed f16 SBUF so
                        # every DVE op runs the 2-byte fast path; DVE also
                        # never reads two PSUM operands this way.
                        cs = c_st[d]
                        sg = ew.tile([128, 256], F16, tag=f"sg{d}")
                        nc.scalar.activation(sg[:], g[:, SI.start:SF.stop],
                                             AF.Sigmoid)
                        gp = ew.tile([128, 128], F16, tag=f"gp{d}")
                        nc.scalar.activation(gp[:], g[:, SG], AF.Tanh)
                        if t == 0:
                            nc.vector.tensor_mul(cs[:], sg[:, 0:128], gp[:])
                        else:
                            cq = ew.tile([128, 128], F16, tag=f"cq{d}")
                            nc.vector.tensor_mul(cq[:], sg[:, 128:256], cs[:])
                            pp = ew.tile([128, 128], F16, tag=f"pp{d}")
                            nc.vector.tensor_mul(pp[:], sg[:, 0:128], gp[:])
                            nc.vector.tensor_add(cs[:], pp[:], cq[:])
                        so = ew.tile([128, 128], F16, tag=f"so{d}")
                        nc.scalar.activation(so[:], g[:, SO], AF.Sigmoid)
                        tc_ = ew.tile([128, 128], F16, tag=f"tc{d}")
                        nc.scalar.activation(tc_[:], cs[:], AF.Tanh)
                        # write h in two halves so the next step's first Whh
                        # matmuls (kr 0,1) can start before the second half
                        nc.vector.tensor_mul(Hb[d][:, tx, 0:2, :],
                                             so[:, 0:64], tc_[:, 0:64])
                        nc.vector.tensor_mul(Hb[d][:, tx, 2:4, :],
                                             so[:, 64:128], tc_[:, 64:128])
                        if t + 1 < T:
                            gcur[d] = x_accum(t + 1, d)

            # ---------------- Phase 3: attention + enhancement ----------------
            with tc.tile_pool(name="abuf", bufs=2) as abuf, \
                 tc.tile_pool(name="zbuf", bufs=2) as zbuf, \
                 tc.tile_pool(name="tmps", bufs=1, space="PSUM") as tmps, \
                 tc.tile_pool(name="eps", bufs=1, space="PSUM") as eps_pool, \
                 tc.tile_pool(name="tilps", bufs=1, space="PSUM") as til_pool:
                for n in range(PB):
                    bigs = {}
                    for s, row in (("a", n), ("b", PB + n)):
                        big = abuf.tile([128, OUT], F16, tag=f"big{s}")
                        tm_ps = tmps.tile([128, H2], F16, tag="tm")
                        for c8 in range(8):
                            d, kc = "fb"[c8 // 4], c8 % 4
                            nc.tensor.transpose(
                                tm_ps[:, c8 * 128:(c8 + 1) * 128],
                                Hb[d][:, :, kc, row], ident16[:])
                        nc.vector.tensor_copy(big[:, 0:H2], tm_ps[:])
                        bigs[s] = big
                    e1t = eps_pool.tile([128, 128], F32, tag="e1")
                    e2t = eps_pool.tile([128, 128], F32, tag="e2")
                    e_ps = e1t[:]
                    e2_ps = e2t[:]
                    for c8 in range(8):
                        d, kc = "fb"[c8 // 4], c8 % 4
                        asl = Hb[d][:, :, kc, n]
                        bsl = Hb[d][:, :, kc, PB + n]
                        nc.tensor.matmul(e_ps, asl, bsl,
                                         start=(c8 == 0), stop=(c8 == 7))
                    for c8 in range(8):
                        d, kc = "fb"[c8 // 4], c8 % 4
                        asl = Hb[d][:, :, kc, n]
                        bsl = Hb[d][:, :, kc, PB + n]
                        nc.tensor.matmul(e2_ps, bsl, asl,
                                         start=(c8 == 0), stop=(c8 == 7))
                    for ei, (ep, rhs_s, dst_s) in enumerate(
                            ((e_ps, "b", "a"), (e2_ps, "a", "b"))):
                        m_ = zbuf.tile([128, 1], F32, tag=f"m{ei}")
                        nc.vector.tensor_reduce(m_[:], ep, axis=AX.X,
                                                op=ALU.max, negate=True)
                        z_ = zbuf.tile([128, 128], F16, tag=f"z{ei}")
                        s_ = zbuf.tile([128, 1], F32, tag=f"s{ei}")
                        nc.scalar.activation(z_[:], ep, AF.Exp, bias=m_[:],
                                             accum_out=s_[:])
                        r_ = zbuf.tile([128, 1], F32, tag=f"r{ei}")
                        nc.vector.reciprocal(r_[:], s_[:])
                        ztp = eps_pool.tile([128, 128], F16, tag="ztp")
                        nc.tensor.transpose(ztp[:], z_[:], ident16[:])
                        zt = zbuf.tile([128, 128], F16, tag=f"zt{ei}")
                        nc.vector.tensor_copy(zt[:], ztp[:])
                        til_ps = til_pool.tile([128, H2], F32, tag=f"til{ei}")
                        rhs = bigs[rhs_s]
                        for hh in range(2):
                            sl = slice(512 * hh, 512 * (hh + 1))
                            nc.tensor.matmul(til_ps[:, sl], zt[:], rhs[:, sl],
                                             start=True, stop=True)
                        dst = bigs[dst_s]
                        nc.scalar.activation(dst[:, H2:2 * H2], til_ps[:],
                                             AF.Copy, scale=r_[:])
                        # diff/prod: all-SBUF f16, split halves DVE/Pool
                        nc.vector.tensor_sub(dst[:, 2 * H2:2 * H2 + 512],
                                             dst[:, 0:512], dst[:, H2:H2 + 512])
                        nc.gpsimd.tensor_sub(dst[:, 2 * H2 + 512:3 * H2],
                                             dst[:, 512:H2],
                                             dst[:, H2 + 512:2 * H2])
                        nc.gpsimd.tensor_mul(dst[:, 3 * H2:3 * H2 + 512],
                                             dst[:, 0:512], dst[:, H2:H2 + 512])
                        nc.vector.tensor_mul(dst[:, 3 * H2 + 512:4 * H2],
                                             dst[:, 512:H2],
                                             dst[:, H2 + 512:2 * H2])
                    nc.sync.dma_start(outA_d.ap()[n], bigs["a"][:])
                    nc.sync.dma_start(outB_d.ap()[n], bigs["b"][:])

    nc.compile()
    return nc


def _get_nc():
    if "nc" not in _CACHE:
        _CACHE["nc"] = _build()
    return _CACHE["nc"]


def prep_in_maps(inputs):
    A = np.asarray(inputs["A"])
    B = np.asarray(inputs["B"])
    embed = np.asarray(inputs["embed"], dtype=np.float32)
    # permute pytorch gate order [i,f,g,o] -> [g,i,f,o]
    perm = np.concatenate([np.arange(2 * H, 3 * H), np.arange(0, 2 * H),
                           np.arange(3 * H, 4 * H)])
    wmat = {}
    for d in "fb":
        suf = "_" + d
        wihT = np.asarray(inputs["Wih" + suf], dtype=np.float32)[perm].T
        bias = (np.asarray(inputs["bih" + suf], dtype=np.float32)
                + np.asarray(inputs["bhh" + suf], dtype=np.float32))[perm]
        wihT_aug = np.concatenate([wihT, bias[None, :]], axis=0)  # [301, 2048]
        whhT = np.asarray(inputs["Whh" + suf], dtype=np.float32)[perm].T
        wmat[d] = (np.ascontiguousarray(wihT_aug, dtype=np.float16),
                   np.ascontiguousarray(whhT, dtype=np.float16))

    xa = embed[A]    # [BSZ, T, E]
    xb = embed[B]

    in_maps = []
    for c in range(NCORES):
        sl = slice(PB * c, PB * (c + 1))
        xc = np.concatenate([xa[sl], xb[sl]], axis=0)          # [RW, T, E]
        xT = xc.transpose(2, 1, 0)                             # [E, T, RW]
        xTb = np.concatenate(
            [xT, np.ones((1, T, RW), np.float32)], axis=0)     # [EB, T, RW]
        in_maps.append({
            "xTb": np.ascontiguousarray(xTb, dtype=np.float16),
            "wihT_f": wmat["f"][0], "whhT_f": wmat["f"][1],
            "wihT_b": wmat["b"][0], "whhT_b": wmat["b"][1],
        })
    return in_maps


def kernel(**inputs):
    from concourse.bass_utils import run_bass_kernel_spmd

    in_maps = prep_in_maps(inputs)
    nc = _get_nc()
    res = run_bass_kernel_spmd(nc, in_maps, core_ids=list(range(NCORES)))
    outA = np.concatenate(
        [res.results[c]["outA"] for c in range(NCORES)], axis=0)
    outB = np.concatenate(
        [res.results[c]["outB"] for c in range(NCORES)], axis=0)
    return outA.astype(np.float32), outB.astype(np.float32)


# revision 40
# speedup vs baseline: 3.4026x; 1.1164x over previous
"""Bass/Trainium2 kernel for nn_Encoder (embedding -> BiLSTM -> cross attention
-> enhancement).

Sharding: data-parallel over batch, 16 items per core on 8 NeuronCores (no
collectives). Per core the A and B sequences are stacked into RW=32 rows.

The BiLSTM scan runs FEATURE-MAJOR: all state (gates, c, h) lives in
[128-feature-partition x 32-row] tiles, so the recurrence needs no PE
transposes and every elementwise op uses all 128 partitions. The input
projection x@Wih^T + bias is fused into the gate PSUM accumulation as extra
matmuls against a bias-augmented [301, 2048] weight (xT carries a constant
ones row), issued one step ahead so they fill PE gaps while the current
step's elementwise chain runs. Gates accumulate into TWO psum bank-tiles
per (dir, step) - [i|f] and [g|o] in pytorch gate order - so sigmoid(i,f)
starts after only half the Whh matmuls. ACT writes activated gates to
packed f16 SBUF tiles, putting every cell-update DVE op on the 2-byte
fast path; h is written (in two halves, unblocking the next step's first
Whh matmuls early) as f16 into a persistent SBUF buffer that both the
next step's matmuls and the attention phase consume.

Attention: per item, PE transposes build time-major [T, H2] "bar" tiles from
the SBUF h buffer, E/E^T come from feature-major matmuls, softmax via
Exp-with-accum, soft alignment matmuls, and the 4-way enhancement concat is
assembled in one [128, 4096] f16 tile and written with a single DMA per
sequence. Outputs are f16 in DRAM; the host converts to f32.
"""

import numpy as np

V, E, H = 32000, 300, 512
BSZ, T = 128, 128
NCORES = 8
PB = BSZ // NCORES          # 16 batch items per core
RW = 2 * PB                 # 32 stacked rows (A items then B items)
G4 = 4 * H                  # 2048 gate width
H2 = 2 * H                  # 1024 bilstm output width
EB = E + 1                  # embedding dim + folded bias row
KCH = [(0, 128), (128, 128), (256, EB - 256)]   # chunks of EB=301
OUT = 4 * H2

_CACHE = {}


def _build():
    import concourse.mybir as mybir
    import concourse.tile as tile
    from concourse import bacc
    from concourse.masks import make_identity

    F32 = mybir.dt.float32
    F16 = mybir.dt.float16
    AF = mybir.ActivationFunctionType
    ALU = mybir.AluOpType
    AX = mybir.AxisListType

    nc = bacc.Bacc("TRN2", target_bir_lowering=False, debug=False,
                   num_devices=NCORES)

    xT_d = nc.dram_tensor("xTb", [EB, T, RW], F16, kind="ExternalInput")
    wih_d = {d: nc.dram_tensor(f"wihT_{d}", [EB, G4], F16, kind="ExternalInput")
             for d in "fb"}
    whh_d = {d: nc.dram_tensor(f"whhT_{d}", [H, G4], F16, kind="ExternalInput")
             for d in "fb"}
    outA_d = nc.dram_tensor("outA", [PB, T, OUT], F16, kind="ExternalOutput")
    outB_d = nc.dram_tensor("outB", [PB, T, OUT], F16, kind="ExternalOutput")

    # gates split across two psum bank-tiles: IF=[i|f], GO=[g|o]; weights
    # keep pytorch gate order [i,f,g,o] so fc 0-7 -> IF bank, 8-15 -> GO
    SI = slice(0, 128)
    SF = slice(128, 256)
    SG = slice(0, 128)
    SO = slice(128, 256)

    with tile.TileContext(nc) as tc:
        with tc.tile_pool(name="const", bufs=1) as const, \
             tc.tile_pool(name="hbuf", bufs=1) as hbuf:
            identf = const.tile([128, 128], F32)
            make_identity(nc, identf[:])
            ident16 = const.tile([128, 128], F16)
            nc.vector.tensor_copy(ident16[:], identf[:])
            # persistent h buffers: [feature-in-chunk, t, kc, row], f16
            Hb = {d: hbuf.tile([128, T, 4, RW], F16, name=f"Hb_{d}")
                  for d in "fb"}

            # ---------------- Phase 1+2: fused proj + scan ----------------
            with tc.tile_pool(name="wst", bufs=1) as wst, \
                 tc.tile_pool(name="sst", bufs=1) as sst, \
                 tc.tile_pool(name="ew", bufs=2) as ew, \
                 tc.tile_pool(name="gps", bufs=2, space="PSUM") as gps_pool:
                xT_sb = []
                qs = [nc.sync, nc.scalar, nc.gpsimd]
                qi = 0
                for ki, (ko, ks) in enumerate(KCH):
                    t_ = wst.tile([ks, T, RW], F16, tag=f"xT{ki}")
                    qs[qi % 3].dma_start(t_[:], xT_d.ap()[ko:ko + ks])
                    qi += 1
                    xT_sb.append(t_)
                wih_sb, whh_sb = {}, {}
                for d in "fb":
                    wih_sb[d] = []
                    for ki, (ko, ks) in enumerate(KCH):
                        w = wst.tile([ks, G4], F16, tag=f"wih{d}{ki}")
                        qs[qi % 3].dma_start(w[:], wih_d[d].ap()[ko:ko + ks])
                        qi += 1
                        wih_sb[d].append(w)
                    whh_sb[d] = []
                    for kr in range(4):
                        w = wst.tile([128, G4], F16, tag=f"whh{d}{kr}")
                        qs[qi % 3].dma_start(
                            w[:], whh_d[d].ap()[kr * 128:(kr + 1) * 128])
                        qi += 1
                        whh_sb[d].append(w)
                # c state in f16: every cell-update DVE op is then a packed
                # 2-byte all-SBUF op (half-rate cycles on DVE)
                c_st = {d: sst.tile([128, 128], F16, name=f"c_{d}")
                        for d in "fb"}

                def x_accum(t, d):
                    """Issue x-part (incl bias) matmuls for step t into two
                    fresh psum bank-tiles (IF gates, GO gates). One
                    accumulation group per bank: start=True only on the
                    bank's first matmul, stop=True only on its last (at t==0
                    the last x matmul, else the last Whh matmul)."""
                    tx = t if d == "f" else T - 1 - t
                    gif = gps_pool.tile([128, 512], F32, tag=f"gi{d}",
                                        name=f"gi{d}")
                    ggo = gps_pool.tile([128, 512], F32, tag=f"gg{d}",
                                        name=f"gg{d}")
                    for bank, g in ((0, gif), (1, ggo)):
                        for f8 in range(8):
                            fc = bank * 8 + f8
                            fs = slice(fc * 128, (fc + 1) * 128)
                            for ki in range(3):
                                nc.tensor.matmul(
                                    g[:, f8 * RW:(f8 + 1) * RW],
                                    wih_sb[d][ki][:, fs],
                                    xT_sb[ki][:, tx, :],
                                    start=(f8 == 0 and ki == 0),
                                    stop=(t == 0 and f8 == 7 and ki == 2))
                    return gif, ggo

                gcur = {d: x_accum(0, d) for d in "fb"}

                for t in range(T):
                    for d in "fb":
                        tx = t if d == "f" else T - 1 - t
                        txp = tx - 1 if d == "f" else tx + 1
                        gif, ggo = gcur[d]
                        if t > 0:
                            for bank, g in ((0, gif), (1, ggo)):
                                for f8 in range(8):
                                    fc = bank * 8 + f8
                                    fs = slice(fc * 128, (fc + 1) * 128)
                                    for kr in range(4):
                                        nc.tensor.matmul(
                                            g[:, f8 * RW:(f8 + 1) * RW],
                                            whh_sb[d][kr][:, fs],
                                            Hb[d][:, txp, kr, :],
                                            start=False,
                                            stop=(f8 == 7 and kr == 3))
                        # elementwise, all [128, 128] feature-major. ACT
                        # moves activated gates PSUM -> packed f16 SBUF so
                        # every DVE op runs the 2-byte fast path; DVE also
                        # never reads two PSUM operands this way.
                        cs = c_st[d]
                        sg = ew.tile([128, 256], F16, tag=f"sg{d}")
                        nc.scalar.activation(sg[:], gif[:, 0:256], AF.Sigmoid)
                        gp = ew.tile([128, 128], F16, tag=f"gp{d}")
                        nc.scalar.activation(gp[:], ggo[:, SG], AF.Tanh)
                        if t == 0:
                            nc.vector.tensor_mul(cs[:], sg[:, 0:128], gp[:])
                        else:
                            cq = ew.tile([128, 128], F16, tag=f"cq{d}")
                            nc.vector.tensor_mul(cq[:], sg[:, 128:256], cs[:])
                            pp = ew.tile([128, 128], F16, tag=f"pp{d}")
                            nc.vector.tensor_mul(pp[:], sg[:, 0:128], gp[:])
                            nc.vector.tensor_add(cs[:], pp[:], cq[:])
                        so = ew.tile([128, 128], F16, tag=f"so{d}")
                        nc.scalar.activation(so[:], ggo[:, SO], AF.Sigmoid)
                        tc_ = ew.tile([128, 128], F16, tag=f"tc{d}")
                        nc.scalar.activation(tc_[:], cs[:], AF.Tanh)
                        # write h in two halves so the next step's first Whh
                        # matmuls (kr 0,1) can start before the second half
                        nc.vector.tensor_mul(Hb[d][:, tx, 0:2, :],
                                             so[:, 0:64], tc_[:, 0:64])
                        nc.vector.tensor_mul(Hb[d][:, tx, 2:4, :],
                                             so[:, 64:128], tc_[:, 64:128])
                        if t + 1 < T:
                            gcur[d] = x_accum(t + 1, d)

            # ---------------- Phase 3: attention + enhancement ----------------
            with tc.tile_pool(name="abuf", bufs=2) as abuf, \
                 tc.tile_pool(name="zbuf", bufs=2) as zbuf, \
                 tc.tile_pool(name="tmps", bufs=1, space="PSUM") as tmps, \
                 tc.tile_pool(name="eps", bufs=1, space="PSUM") as eps_pool, \
                 tc.tile_pool(name="tilps", bufs=1, space="PSUM") as til_pool:
                for n in range(PB):
                    bigs = {}
                    for s, row in (("a", n), ("b", PB + n)):
                        big = abuf.tile([128, OUT], F16, tag=f"big{s}")
                        tm_ps = tmps.tile([128, H2], F16, tag="tm")
                        for c8 in range(8):
                            d, kc = "fb"[c8 // 4], c8 % 4
                            nc.tensor.transpose(
                                tm_ps[:, c8 * 128:(c8 + 1) * 128],
                                Hb[d][:, :, kc, row], ident16[:])
                        nc.vector.tensor_copy(big[:, 0:H2], tm_ps[:])
                        bigs[s] = big
                    e1t = eps_pool.tile([128, 128], F32, tag="e1")
                    e2t = eps_pool.tile([128, 128], F32, tag="e2")
                    e_ps = e1t[:]
                    e2_ps = e2t[:]
                    for c8 in range(8):
                        d, kc = "fb"[c8 // 4], c8 % 4
                        asl = Hb[d][:, :, kc, n]
                        bsl = Hb[d][:, :, kc, PB + n]
                        nc.tensor.matmul(e_ps, asl, bsl,
                                         start=(c8 == 0), stop=(c8 == 7))
                    for c8 in range(8):
                        d, kc = "fb"[c8 // 4], c8 % 4
                        asl = Hb[d][:, :, kc, n]
                        bsl = Hb[d][:, :, kc, PB + n]
                        nc.tensor.matmul(e2_ps, bsl, asl,
                                         start=(c8 == 0), stop=(c8 == 7))
                    for ei, (ep, rhs_s, dst_s) in enumerate(
                            ((e_ps, "b", "a"), (e2_ps, "a", "b"))):
                        m_ = zbuf.tile([128, 1], F32, tag=f"m{ei}")
                        nc.vector.tensor_reduce(m_[:], ep, axis=AX.X,
                                                op=ALU.max, negate=True)
                        z_ = zbuf.tile([128, 128], F16, tag=f"z{ei}")
                        s_ = zbuf.tile([128, 1], F32, tag=f"s{ei}")
                        nc.scalar.activation(z_[:], ep, AF.Exp, bias=m_[:],
                                             accum_out=s_[:])
                        r_ = zbuf.tile([128, 1], F32, tag=f"r{ei}")
                        nc.vector.reciprocal(r_[:], s_[:])
                        ztp = eps_pool.tile([128, 128], F16, tag="ztp")
                        nc.tensor.transpose(ztp[:], z_[:], ident16[:])
                        zt = zbuf.tile([128, 128], F16, tag=f"zt{ei}")
                        nc.vector.tensor_copy(zt[:], ztp[:])
                        til_ps = til_pool.tile([128, H2], F32, tag=f"til{ei}")
                        rhs = bigs[rhs_s]
                        for hh in range(2):
                            sl = slice(512 * hh, 512 * (hh + 1))
                            nc.tensor.matmul(til_ps[:, sl], zt[:], rhs[:, sl],
                                             start=True, stop=True)
                        dst = bigs[dst_s]
                        nc.scalar.activation(dst[:, H2:2 * H2], til_ps[:],
                                             AF.Copy, scale=r_[:])
                        # diff/prod: all-SBUF f16, split halves DVE/Pool
                        nc.vector.tensor_sub(dst[:, 2 * H2:2 * H2 + 512],
                                             dst[:, 0:512], dst[:, H2:H2 + 512])
                        nc.gpsimd.tensor_sub(dst[:, 2 * H2 + 512:3 * H2],
                                             dst[:, 512:H2],
                                             dst[:, H2 + 512:2 * H2])
                        nc.gpsimd.tensor_mul(dst[:, 3 * H2:3 * H2 + 512],
                                             dst[:, 0:512], dst[:, H2:H2 + 512])
                        nc.vector.tensor_mul(dst[:, 3 * H2 + 512:4 * H2],
                                             dst[:, 512:H2],
                                             dst[:, H2 + 512:2 * H2])
                    nc.sync.dma_start(outA_d.ap()[n], bigs["a"][:])
                    nc.sync.dma_start(outB_d.ap()[n], bigs["b"][:])

    nc.compile()
    return nc


def _get_nc():
    if "nc" not in _CACHE:
        _CACHE["nc"] = _build()
    return _CACHE["nc"]


def prep_in_maps(inputs):
    A = np.asarray(inputs["A"])
    B = np.asarray(inputs["B"])
    embed = np.asarray(inputs["embed"], dtype=np.float32)
    # gates stay in pytorch order [i,f,g,o]: fc 0-7 feed the IF psum bank,
    # fc 8-15 the GO bank
    perm = np.arange(4 * H)
    wmat = {}
    for d in "fb":
        suf = "_" + d
        wihT = np.asarray(inputs["Wih" + suf], dtype=np.float32)[perm].T
        bias = (np.asarray(inputs["bih" + suf], dtype=np.float32)
                + np.asarray(inputs["bhh" + suf], dtype=np.float32))[perm]
        wihT_aug = np.concatenate([wihT, bias[None, :]], axis=0)  # [301, 2048]
        whhT = np.asarray(inputs["Whh" + suf], dtype=np.float32)[perm].T
        wmat[d] = (np.ascontiguousarray(wihT_aug, dtype=np.float16),
                   np.ascontiguousarray(whhT, dtype=np.float16))

    xa = embed[A]    # [BSZ, T, E]
    xb = embed[B]

    in_maps = []
    for c in range(NCORES):
        sl = slice(PB * c, PB * (c + 1))
        xc = np.concatenate([xa[sl], xb[sl]], axis=0)          # [RW, T, E]
        xT = xc.transpose(2, 1, 0)                             # [E, T, RW]
        xTb = np.concatenate(
            [xT, np.ones((1, T, RW), np.float32)], axis=0)     # [EB, T, RW]
        in_maps.append({
            "xTb": np.ascontiguousarray(xTb, dtype=np.float16),
            "wihT_f": wmat["f"][0], "whhT_f": wmat["f"][1],
            "wihT_b": wmat["b"][0], "whhT_b": wmat["b"][1],
        })
    return in_maps


def kernel(**inputs):
    from concourse.bass_utils import run_bass_kernel_spmd

    in_maps = prep_in_maps(inputs)
    nc = _get_nc()
    res = run_bass_kernel_spmd(nc, in_maps, core_ids=list(range(NCORES)))
    outA = np.concatenate(
        [res.results[c]["outA"] for c in range(NCORES)], axis=0)
    outB = np.concatenate(
        [res.results[c]["outB"] for c in range(NCORES)], axis=0)
    return outA.astype(np.float32), outB.astype(np.float32)


# revision 44
# speedup vs baseline: 3.4737x; 1.0209x over previous
"""Bass/Trainium2 kernel for nn_Encoder (embedding -> BiLSTM -> cross attention
-> enhancement).

Sharding: data-parallel over batch, 16 items per core on 8 NeuronCores (no
collectives). Per core the A and B sequences are stacked into RW=32 rows.

The BiLSTM scan runs FEATURE-MAJOR: all state (gates, c, h) lives in
[128-feature-partition x 32-row] tiles, so the recurrence needs no PE
transposes and every elementwise op uses all 128 partitions. The input
projection x@Wih^T + bias is fused into the gate PSUM accumulation as extra
matmuls against a bias-augmented [301, 2048] weight (xT carries a constant
ones row), issued one step ahead so they fill PE gaps while the current
step's elementwise chain runs. Gates accumulate into TWO psum bank-tiles
per (dir, step) - [i|f] and [g|o] in pytorch gate order - so sigmoid(i,f)
starts after only half the Whh matmuls. ACT writes activated gates to
packed f16 SBUF tiles, putting every cell-update DVE op on the 2-byte
fast path; h is written (in two halves, unblocking the next step's first
Whh matmuls early) as f16 into a persistent SBUF buffer that both the
next step's matmuls and the attention phase consume.

Attention: per item, PE transposes build time-major [T, H2] "bar" tiles from
the SBUF h buffer, E/E^T come from feature-major matmuls, softmax via
Exp-with-accum, soft alignment matmuls, and the 4-way enhancement concat is
assembled in one [128, 4096] f16 tile and written with a single DMA per
sequence. Outputs are f16 in DRAM; the host converts to f32.
"""

import numpy as np

V, E, H = 32000, 300, 512
BSZ, T = 128, 128
NCORES = 8
PB = BSZ // NCORES          # 16 batch items per core
RW = 2 * PB                 # 32 stacked rows (A items then B items)
G4 = 4 * H                  # 2048 gate width
H2 = 2 * H                  # 1024 bilstm output width
EB = E + 1                  # embedding dim + folded bias row
KCH = [(0, 128), (128, 128), (256, EB - 256)]   # chunks of EB=301
OUT = 4 * H2

_CACHE = {}


def _build():
    import concourse.mybir as mybir
    import concourse.tile as tile
    from concourse import bacc
    from concourse.masks import make_identity

    F32 = mybir.dt.float32
    F16 = mybir.dt.float16
    AF = mybir.ActivationFunctionType
    ALU = mybir.AluOpType
    AX = mybir.AxisListType

    nc = bacc.Bacc("TRN2", target_bir_lowering=False, debug=False,
                   num_devices=NCORES)

    xT_d = nc.dram_tensor("xTb", [EB, T, RW], F16, kind="ExternalInput")
    wih_d = {d: nc.dram_tensor(f"wihT_{d}", [EB, G4], F16, kind="ExternalInput")
             for d in "fb"}
    whh_d = {d: nc.dram_tensor(f"whhT_{d}", [H, G4], F16, kind="ExternalInput")
             for d in "fb"}
    outA_d = nc.dram_tensor("outA", [PB, T, OUT], F16, kind="ExternalOutput")
    outB_d = nc.dram_tensor("outB", [PB, T, OUT], F16, kind="ExternalOutput")

    # gates split across two psum bank-tiles: IF=[i|f], GO=[g|o]; weights
    # keep pytorch gate order [i,f,g,o] so fc 0-7 -> IF bank, 8-15 -> GO
    SI = slice(0, 128)
    SF = slice(128, 256)
    SG = slice(0, 128)
    SO = slice(128, 256)

    with tile.TileContext(nc) as tc:
        with tc.tile_pool(name="const", bufs=1) as const, \
             tc.tile_pool(name="hbuf", bufs=1) as hbuf:
            identf = const.tile([128, 128], F32)
            make_identity(nc, identf[:])
            ident16 = const.tile([128, 128], F16)
            nc.vector.tensor_copy(ident16[:], identf[:])
            # persistent h buffers: [feature-in-chunk, t, kc, row], f16
            Hb = {d: hbuf.tile([128, T, 4, RW], F16, name=f"Hb_{d}")
                  for d in "fb"}

            # ---------------- Phase 1+2: fused proj + scan ----------------
            with tc.tile_pool(name="wst", bufs=1) as wst, \
                 tc.tile_pool(name="sst", bufs=1) as sst, \
                 tc.tile_pool(name="ew", bufs=2) as ew, \
                 tc.tile_pool(name="gps", bufs=2, space="PSUM") as gps_pool:
                xT_sb = []
                qs = [nc.sync, nc.scalar, nc.gpsimd]
                qi = 0
                for ki, (ko, ks) in enumerate(KCH):
                    t_ = wst.tile([ks, T, RW], F16, tag=f"xT{ki}")
                    qs[qi % 3].dma_start(t_[:], xT_d.ap()[ko:ko + ks])
                    qi += 1
                    xT_sb.append(t_)
                wih_sb, whh_sb = {}, {}
                for d in "fb":
                    wih_sb[d] = []
                    for ki, (ko, ks) in enumerate(KCH):
                        w = wst.tile([ks, G4], F16, tag=f"wih{d}{ki}")
                        qs[qi % 3].dma_start(w[:], wih_d[d].ap()[ko:ko + ks])
                        qi += 1
                        wih_sb[d].append(w)
                    whh_sb[d] = []
                    for kr in range(4):
                        w = wst.tile([128, G4], F16, tag=f"whh{d}{kr}")
                        qs[qi % 3].dma_start(
                            w[:], whh_d[d].ap()[kr * 128:(kr + 1) * 128])
                        qi += 1
                        whh_sb[d].append(w)
                # c state in f16: every cell-update DVE op is then a packed
                # 2-byte all-SBUF op (half-rate cycles on DVE)
                c_st = {d: sst.tile([128, 128], F16, name=f"c_{d}")
                        for d in "fb"}

                def x_accum(t, d):
                    """Issue x-part (incl bias) matmuls for step t into two
                    fresh psum bank-tiles (IF gates, GO gates). One
                    accumulation group per bank: start=True only on the
                    bank's first matmul, stop=True only on its last (at t==0
                    the last x matmul, else the last Whh matmul)."""
                    tx = t if d == "f" else T - 1 - t
                    gif = gps_pool.tile([128, 512], F32, tag=f"gi{d}",
                                        name=f"gi{d}")
                    ggo = gps_pool.tile([128, 512], F32, tag=f"gg{d}",
                                        name=f"gg{d}")
                    for bank, g in ((0, gif), (1, ggo)):
                        for f8 in range(8):
                            fc = bank * 8 + f8
                            fs = slice(fc * 128, (fc + 1) * 128)
                            for ki in range(3):
                                nc.tensor.matmul(
                                    g[:, f8 * RW:(f8 + 1) * RW],
                                    wih_sb[d][ki][:, fs],
                                    xT_sb[ki][:, tx, :],
                                    start=(f8 == 0 and ki == 0),
                                    stop=(t == 0 and f8 == 7 and ki == 2))
                    return gif, ggo

                gcur = {d: x_accum(0, d) for d in "fb"}

                for t in range(T):
                    for d in "fb":
                        tx = t if d == "f" else T - 1 - t
                        txp = tx - 1 if d == "f" else tx + 1
                        gif, ggo = gcur[d]
                        if t > 0:
                            for bank, g in ((0, gif), (1, ggo)):
                                for f8 in range(8):
                                    fc = bank * 8 + f8
                                    fs = slice(fc * 128, (fc + 1) * 128)
                                    for kr in range(4):
                                        nc.tensor.matmul(
                                            g[:, f8 * RW:(f8 + 1) * RW],
                                            whh_sb[d][kr][:, fs],
                                            Hb[d][:, txp, kr, :],
                                            start=False,
                                            stop=(f8 == 7 and kr == 3))
                        # elementwise, all [128, 128] feature-major. ACT
                        # moves activated gates PSUM -> packed f16 SBUF so
                        # every DVE op runs the 2-byte fast path; DVE also
                        # never reads two PSUM operands this way.
                        cs = c_st[d]
                        sg = ew.tile([128, 256], F16, tag=f"sg{d}")
                        nc.scalar.activation(sg[:], gif[:, 0:256], AF.Sigmoid)
                        gp = ew.tile([128, 128], F16, tag=f"gp{d}")
                        nc.scalar.activation(gp[:], ggo[:, SG], AF.Tanh)
                        if t == 0:
                            nc.vector.tensor_mul(cs[:], sg[:, 0:128], gp[:])
                        else:
                            cq = ew.tile([128, 128], F16, tag=f"cq{d}")
                            nc.vector.tensor_mul(cq[:], sg[:, 128:256], cs[:])
                            pp = ew.tile([128, 128], F16, tag=f"pp{d}")
                            nc.vector.tensor_mul(pp[:], sg[:, 0:128], gp[:])
                            nc.vector.tensor_add(cs[:], pp[:], cq[:])
                        so = ew.tile([128, 128], F16, tag=f"so{d}")
                        nc.scalar.activation(so[:], ggo[:, SO], AF.Sigmoid)
                        tc_ = ew.tile([128, 128], F16, tag=f"tc{d}")
                        nc.scalar.activation(tc_[:], cs[:], AF.Tanh)
                        # write h in two halves so the next step's first Whh
                        # matmuls (kr 0,1) can start before the second half
                        nc.vector.tensor_mul(Hb[d][:, tx, 0:2, :],
                                             so[:, 0:64], tc_[:, 0:64])
                        nc.vector.tensor_mul(Hb[d][:, tx, 2:4, :],
                                             so[:, 64:128], tc_[:, 64:128])
                        if t + 1 < T:
                            gcur[d] = x_accum(t + 1, d)

            # ---------------- Phase 3: attention + enhancement ----------------
            with tc.tile_pool(name="abuf", bufs=2) as abuf, \
                 tc.tile_pool(name="zbuf", bufs=2) as zbuf, \
                 tc.tile_pool(name="tmps", bufs=1, space="PSUM") as tmps, \
                 tc.tile_pool(name="eps", bufs=1, space="PSUM") as eps_pool, \
                 tc.tile_pool(name="tilps", bufs=1, space="PSUM") as til_pool:
                for n in range(PB):
                    bigs = {}
                    for s, row in (("a", n), ("b", PB + n)):
                        big = abuf.tile([128, OUT], F16, tag=f"big{s}")
                        tm_ps = tmps.tile([128, H2], F16, tag="tm")
                        for c8 in range(8):
                            d, kc = "fb"[c8 // 4], c8 % 4
                            nc.tensor.transpose(
                                tm_ps[:, c8 * 128:(c8 + 1) * 128],
                                Hb[d][:, :, kc, row], ident16[:])
                        nc.vector.tensor_copy(big[:, 0:H2], tm_ps[:])
                        bigs[s] = big
                    e1t = eps_pool.tile([128, 128], F32, tag="e1")
                    e2t = eps_pool.tile([128, 128], F32, tag="e2")
                    e_ps = e1t[:]
                    e2_ps = e2t[:]
                    for c8 in range(8):
                        d, kc = "fb"[c8 // 4], c8 % 4
                        asl = Hb[d][:, :, kc, n]
                        bsl = Hb[d][:, :, kc, PB + n]
                        nc.tensor.matmul(e_ps, asl, bsl,
                                         start=(c8 == 0), stop=(c8 == 7))
                    for c8 in range(8):
                        d, kc = "fb"[c8 // 4], c8 % 4
                        asl = Hb[d][:, :, kc, n]
                        bsl = Hb[d][:, :, kc, PB + n]
                        nc.tensor.matmul(e2_ps, bsl, asl,
                                         start=(c8 == 0), stop=(c8 == 7))
                    for ei, (ep, rhs_s, dst_s) in enumerate(
                            ((e_ps, "b", "a"), (e2_ps, "a", "b"))):
                        m_ = zbuf.tile([128, 1], F32, tag=f"m{ei}")
                        nc.vector.tensor_reduce(m_[:], ep, axis=AX.X,
                                                op=ALU.max, negate=True)
                        z_ = zbuf.tile([128, 128], F16, tag=f"z{ei}")
                        s_ = zbuf.tile([128, 1], F32, tag=f"s{ei}")
                        nc.scalar.activation(z_[:], ep, AF.Exp, bias=m_[:],
                                             accum_out=s_[:])
                        r_ = zbuf.tile([128, 1], F32, tag=f"r{ei}")
                        nc.vector.reciprocal(r_[:], s_[:])
                        ztp = eps_pool.tile([128, 128], F16, tag="ztp")
                        nc.tensor.transpose(ztp[:], z_[:], ident16[:])
                        zt = zbuf.tile([128, 128], F16, tag=f"zt{ei}")
                        nc.vector.tensor_copy(zt[:], ztp[:])
                        til_ps = til_pool.tile([128, H2], F32, tag=f"til{ei}")
                        rhs = bigs[rhs_s]
                        for hh in range(2):
                            sl = slice(512 * hh, 512 * (hh + 1))
                            nc.tensor.matmul(til_ps[:, sl], zt[:], rhs[:, sl],
                                             start=True, stop=True)
                        dst = bigs[dst_s]
                        nc.scalar.activation(dst[:, H2:2 * H2], til_ps[:],
                                             AF.Copy, scale=r_[:])
                        # diff/prod: all-SBUF f16, split halves DVE/Pool
                        nc.vector.tensor_sub(dst[:, 2 * H2:2 * H2 + 512],
                                             dst[:, 0:512], dst[:, H2:H2 + 512])
                        nc.gpsimd.tensor_sub(dst[:, 2 * H2 + 512:3 * H2],
                                             dst[:, 512:H2],
                                             dst[:, H2 + 512:2 * H2])
                        nc.gpsimd.tensor_mul(dst[:, 3 * H2:3 * H2 + 512],
                                             dst[:, 0:512], dst[:, H2:H2 + 512])
                        nc.vector.tensor_mul(dst[:, 3 * H2 + 512:4 * H2],
                                             dst[:, 512:H2],
                                             dst[:, H2 + 512:2 * H2])
                    # first half (bar|til) ships while diff/prod compute
                    nc.sync.dma_start(outA_d.ap()[n, :, 0:2 * H2],
                                      bigs["a"][:, 0:2 * H2])
                    nc.sync.dma_start(outB_d.ap()[n, :, 0:2 * H2],
                                      bigs["b"][:, 0:2 * H2])
                    nc.sync.dma_start(outA_d.ap()[n, :, 2 * H2:OUT],
                                      bigs["a"][:, 2 * H2:OUT])
                    nc.sync.dma_start(outB_d.ap()[n, :, 2 * H2:OUT],
                                      bigs["b"][:, 2 * H2:OUT])

    nc.compile()
    return nc


def _get_nc():
    if "nc" not in _CACHE:
        _CACHE["nc"] = _build()
    return _CACHE["nc"]


def prep_in_maps(inputs):
    A = np.asarray(inputs["A"])
    B = np.asarray(inputs["B"])
    embed = np.asarray(inputs["embed"], dtype=np.float32)
    # gates stay in pytorch order [i,f,g,o]: fc 0-7 feed the IF psum bank,
    # fc 8-15 the GO bank
    perm = np.arange(4 * H)
    wmat = {}
    for d in "fb":
        suf = "_" + d
        wihT = np.asarray(inputs["Wih" + suf], dtype=np.float32)[perm].T
        bias = (np.asarray(inputs["bih" + suf], dtype=np.float32)
                + np.asarray(inputs["bhh" + suf], dtype=np.float32))[perm]
        wihT_aug = np.concatenate([wihT, bias[None, :]], axis=0)  # [301, 2048]
        whhT = np.asarray(inputs["Whh" + suf], dtype=np.float32)[perm].T
        wmat[d] = (np.ascontiguousarray(wihT_aug, dtype=np.float16),
                   np.ascontiguousarray(whhT, dtype=np.float16))

    xa = embed[A]    # [BSZ, T, E]
    xb = embed[B]

    in_maps = []
    for c in range(NCORES):
        sl = slice(PB * c, PB * (c + 1))
        xc = np.concatenate([xa[sl], xb[sl]], axis=0)          # [RW, T, E]
        xT = xc.transpose(2, 1, 0)                             # [E, T, RW]
        xTb = np.concatenate(
            [xT, np.ones((1, T, RW), np.float32)], axis=0)     # [EB, T, RW]
        in_maps.append({
            "xTb": np.ascontiguousarray(xTb, dtype=np.float16),
            "wihT_f": wmat["f"][0], "whhT_f": wmat["f"][1],
            "wihT_b": wmat["b"][0], "whhT_b": wmat["b"][1],
        })
    return in_maps


def kernel(**inputs):
    from concourse.bass_utils import run_bass_kernel_spmd

    in_maps = prep_in_maps(inputs)
    nc = _get_nc()
    res = run_bass_kernel_spmd(nc, in_maps, core_ids=list(range(NCORES)))
    outA = np.concatenate(
        [res.results[c]["outA"] for c in range(NCORES)], axis=0)
    outB = np.concatenate(
        [res.results[c]["outB"] for c in range(NCORES)], axis=0)
    return outA.astype(np.float32), outB.astype(np.float32)


# revision 47
# speedup vs baseline: 3.5282x; 1.0157x over previous
"""Bass/Trainium2 kernel for nn_Encoder (embedding -> BiLSTM -> cross attention
-> enhancement).

Sharding: data-parallel over batch, 16 items per core on 8 NeuronCores (no
collectives). Per core the A and B sequences are stacked into RW=32 rows.

The BiLSTM scan runs FEATURE-MAJOR: all state (gates, c, h) lives in
[128-feature-partition x 32-row] tiles, so the recurrence needs no PE
transposes and every elementwise op uses all 128 partitions. The input
projection x@Wih^T + bias is fused into the gate PSUM accumulation as extra
matmuls against a bias-augmented [301, 2048] weight (xT carries a constant
ones row), issued one step ahead so they fill PE gaps while the current
step's elementwise chain runs. Gates accumulate into TWO psum bank-tiles
per (dir, step) - [i|f] and [g|o] in pytorch gate order - so sigmoid(i,f)
starts after only half the Whh matmuls. ACT writes activated gates to
packed f16 SBUF tiles, putting every cell-update DVE op on the 2-byte
fast path; h is written (in two halves, unblocking the next step's first
Whh matmuls early) as f16 into a persistent SBUF buffer that both the
next step's matmuls and the attention phase consume.

Attention: per item, PE transposes build time-major [T, H2] "bar" tiles from
the SBUF h buffer, E/E^T come from feature-major matmuls, softmax via
Exp-with-accum, soft alignment matmuls, and the 4-way enhancement concat is
assembled in one [128, 4096] f16 tile and written with a single DMA per
sequence. Outputs are f16 in DRAM; the host converts to f32.
"""

import numpy as np

V, E, H = 32000, 300, 512
BSZ, T = 128, 128
NCORES = 8
PB = BSZ // NCORES          # 16 batch items per core
RW = 2 * PB                 # 32 stacked rows (A items then B items)
G4 = 4 * H                  # 2048 gate width
H2 = 2 * H                  # 1024 bilstm output width
EB = E + 1                  # embedding dim + folded bias row
KCH = [(0, 128), (128, 128), (256, EB - 256)]   # chunks of EB=301
OUT = 4 * H2

_CACHE = {}


def _build():
    import concourse.mybir as mybir
    import concourse.tile as tile
    from concourse import bacc
    from concourse.masks import make_identity

    F32 = mybir.dt.float32
    F16 = mybir.dt.float16
    AF = mybir.ActivationFunctionType
    ALU = mybir.AluOpType
    AX = mybir.AxisListType

    nc = bacc.Bacc("TRN2", target_bir_lowering=False, debug=False,
                   num_devices=NCORES)

    xT_d = nc.dram_tensor("xTb", [EB, T, RW], F16, kind="ExternalInput")
    wih_d = {d: nc.dram_tensor(f"wihT_{d}", [EB, G4], F16, kind="ExternalInput")
             for d in "fb"}
    whh_d = {d: nc.dram_tensor(f"whhT_{d}", [H, G4], F16, kind="ExternalInput")
             for d in "fb"}
    outA_d = nc.dram_tensor("outA", [PB, T, OUT], F16, kind="ExternalOutput")
    outB_d = nc.dram_tensor("outB", [PB, T, OUT], F16, kind="ExternalOutput")

    # gates split across two psum bank-tiles: IF=[i|f], GO=[g|o]; weights
    # keep pytorch gate order [i,f,g,o] so fc 0-7 -> IF bank, 8-15 -> GO
    SI = slice(0, 128)
    SF = slice(128, 256)
    SG = slice(0, 128)
    SO = slice(128, 256)

    with tile.TileContext(nc) as tc:
        with tc.tile_pool(name="const", bufs=1) as const, \
             tc.tile_pool(name="hbuf", bufs=1) as hbuf:
            identf = const.tile([128, 128], F32)
            make_identity(nc, identf[:])
            ident16 = const.tile([128, 128], F16)
            nc.vector.tensor_copy(ident16[:], identf[:])
            # persistent h buffers: [feature-in-chunk, t, kc, row], f16
            Hb = {d: hbuf.tile([128, T, 4, RW], F16, name=f"Hb_{d}")
                  for d in "fb"}

            # ---------------- Phase 1+2: fused proj + scan ----------------
            with tc.tile_pool(name="wst", bufs=1) as wst, \
                 tc.tile_pool(name="sst", bufs=1) as sst, \
                 tc.tile_pool(name="ew", bufs=2) as ew, \
                 tc.tile_pool(name="gps", bufs=2, space="PSUM") as gps_pool:
                xT_sb = []
                qs = [nc.sync, nc.scalar, nc.gpsimd]
                qi = 0
                for ki, (ko, ks) in enumerate(KCH):
                    t_ = wst.tile([ks, T, RW], F16, tag=f"xT{ki}")
                    qs[qi % 3].dma_start(t_[:], xT_d.ap()[ko:ko + ks])
                    qi += 1
                    xT_sb.append(t_)
                wih_sb, whh_sb = {}, {}
                for d in "fb":
                    wih_sb[d] = []
                    for ki, (ko, ks) in enumerate(KCH):
                        w = wst.tile([ks, G4], F16, tag=f"wih{d}{ki}")
                        qs[qi % 3].dma_start(w[:], wih_d[d].ap()[ko:ko + ks])
                        qi += 1
                        wih_sb[d].append(w)
                    whh_sb[d] = []
                    for kr in range(4):
                        w = wst.tile([128, G4], F16, tag=f"whh{d}{kr}")
                        qs[qi % 3].dma_start(
                            w[:], whh_d[d].ap()[kr * 128:(kr + 1) * 128])
                        qi += 1
                        whh_sb[d].append(w)
                # c state in f16: every cell-update DVE op is then a packed
                # 2-byte all-SBUF op (half-rate cycles on DVE)
                c_st = {d: sst.tile([128, 128], F16, name=f"c_{d}")
                        for d in "fb"}

                def x_accum(t, d):
                    """Issue x-part (incl bias) matmuls for step t into two
                    fresh psum bank-tiles (IF gates, GO gates). One
                    accumulation group per bank: start=True only on the
                    bank's first matmul, stop=True only on its last (at t==0
                    the last x matmul, else the last Whh matmul)."""
                    tx = t if d == "f" else T - 1 - t
                    gif = gps_pool.tile([128, 512], F32, tag=f"gi{d}",
                                        name=f"gi{d}")
                    ggo = gps_pool.tile([128, 512], F32, tag=f"gg{d}",
                                        name=f"gg{d}")
                    for bank, g in ((0, gif), (1, ggo)):
                        for f8 in range(8):
                            fc = bank * 8 + f8
                            fs = slice(fc * 128, (fc + 1) * 128)
                            for ki in range(3):
                                nc.tensor.matmul(
                                    g[:, f8 * RW:(f8 + 1) * RW],
                                    wih_sb[d][ki][:, fs],
                                    xT_sb[ki][:, tx, :],
                                    start=(f8 == 0 and ki == 0),
                                    stop=(t == 0 and f8 == 7 and ki == 2))
                    return gif, ggo

                gcur = {d: x_accum(0, d) for d in "fb"}

                for t in range(T):
                    for d in "fb":
                        tx = t if d == "f" else T - 1 - t
                        txp = tx - 1 if d == "f" else tx + 1
                        gif, ggo = gcur[d]
                        if t > 0:
                            for bank, g in ((0, gif), (1, ggo)):
                                for f8 in range(8):
                                    fc = bank * 8 + f8
                                    fs = slice(fc * 128, (fc + 1) * 128)
                                    for kr in range(4):
                                        nc.tensor.matmul(
                                            g[:, f8 * RW:(f8 + 1) * RW],
                                            whh_sb[d][kr][:, fs],
                                            Hb[d][:, txp, kr, :],
                                            start=False,
                                            stop=(f8 == 7 and kr == 3))
                        # elementwise, all [128, 128] feature-major. ACT
                        # moves activated gates PSUM -> packed f16 SBUF so
                        # every DVE op runs the 2-byte fast path; DVE also
                        # never reads two PSUM operands this way.
                        cs = c_st[d]
                        sg = ew.tile([128, 256], F16, tag=f"sg{d}")
                        nc.scalar.activation(sg[:], gif[:, 0:256], AF.Sigmoid)
                        gp = ew.tile([128, 128], F16, tag=f"gp{d}")
                        nc.scalar.activation(gp[:], ggo[:, SG], AF.Tanh)
                        if t == 0:
                            nc.vector.tensor_mul(cs[:], sg[:, 0:128], gp[:])
                        else:
                            cq = ew.tile([128, 128], F16, tag=f"cq{d}")
                            nc.vector.tensor_mul(cq[:], sg[:, 128:256], cs[:])
                            pp = ew.tile([128, 128], F16, tag=f"pp{d}")
                            nc.vector.tensor_mul(pp[:], sg[:, 0:128], gp[:])
                            nc.vector.tensor_add(cs[:], pp[:], cq[:])
                        so = ew.tile([128, 128], F16, tag=f"so{d}")
                        nc.scalar.activation(so[:], ggo[:, SO], AF.Sigmoid)
                        tc_ = ew.tile([128, 128], F16, tag=f"tc{d}")
                        nc.scalar.activation(tc_[:], cs[:], AF.Tanh)
                        # write h in two halves so the next step's first Whh
                        # matmuls (kr 0,1) can start before the second half
                        nc.vector.tensor_mul(Hb[d][:, tx, 0:2, :],
                                             so[:, 0:64], tc_[:, 0:64])
                        nc.vector.tensor_mul(Hb[d][:, tx, 2:4, :],
                                             so[:, 64:128], tc_[:, 64:128])
                        if t + 1 < T:
                            gcur[d] = x_accum(t + 1, d)

            # ---------------- Phase 3: attention + enhancement ----------------
            with tc.tile_pool(name="abuf", bufs=3) as abuf, \
                 tc.tile_pool(name="zbuf", bufs=2) as zbuf, \
                 tc.tile_pool(name="tmps", bufs=1, space="PSUM") as tmps, \
                 tc.tile_pool(name="eps", bufs=1, space="PSUM") as eps_pool, \
                 tc.tile_pool(name="tilps", bufs=1, space="PSUM") as til_pool:
                for n in range(PB):
                    bigs = {}
                    for s, row in (("a", n), ("b", PB + n)):
                        big = abuf.tile([128, OUT], F16, tag=f"big{s}")
                        tm_ps = tmps.tile([128, H2], F16, tag="tm")
                        for c8 in range(8):
                            d, kc = "fb"[c8 // 4], c8 % 4
                            nc.tensor.transpose(
                                tm_ps[:, c8 * 128:(c8 + 1) * 128],
                                Hb[d][:, :, kc, row], ident16[:])
                        nc.vector.tensor_copy(big[:, 0:H2], tm_ps[:])
                        bigs[s] = big
                    e1t = eps_pool.tile([128, 128], F32, tag="e1")
                    e2t = eps_pool.tile([128, 128], F32, tag="e2")
                    e_ps = e1t[:]
                    e2_ps = e2t[:]
                    for c8 in range(8):
                        d, kc = "fb"[c8 // 4], c8 % 4
                        asl = Hb[d][:, :, kc, n]
                        bsl = Hb[d][:, :, kc, PB + n]
                        nc.tensor.matmul(e_ps, asl, bsl,
                                         start=(c8 == 0), stop=(c8 == 7))
                    for c8 in range(8):
                        d, kc = "fb"[c8 // 4], c8 % 4
                        asl = Hb[d][:, :, kc, n]
                        bsl = Hb[d][:, :, kc, PB + n]
                        nc.tensor.matmul(e2_ps, bsl, asl,
                                         start=(c8 == 0), stop=(c8 == 7))
                    for ei, (ep, rhs_s, dst_s) in enumerate(
                            ((e_ps, "b", "a"), (e2_ps, "a", "b"))):
                        m_ = zbuf.tile([128, 1], F32, tag=f"m{ei}")
                        nc.vector.tensor_reduce(m_[:], ep, axis=AX.X,
                                                op=ALU.max, negate=True)
                        z_ = zbuf.tile([128, 128], F16, tag=f"z{ei}")
                        s_ = zbuf.tile([128, 1], F32, tag=f"s{ei}")
                        nc.scalar.activation(z_[:], ep, AF.Exp, bias=m_[:],
                                             accum_out=s_[:])
                        r_ = zbuf.tile([128, 1], F32, tag=f"r{ei}")
                        nc.vector.reciprocal(r_[:], s_[:])
                        ztp = eps_pool.tile([128, 128], F16, tag="ztp")
                        nc.tensor.transpose(ztp[:], z_[:], ident16[:])
                        zt = zbuf.tile([128, 128], F16, tag=f"zt{ei}")
                        nc.vector.tensor_copy(zt[:], ztp[:])
                        til_ps = til_pool.tile([128, H2], F32, tag=f"til{ei}")
                        rhs = bigs[rhs_s]
                        for hh in range(2):
                            sl = slice(512 * hh, 512 * (hh + 1))
                            nc.tensor.matmul(til_ps[:, sl], zt[:], rhs[:, sl],
                                             start=True, stop=True)
                        dst = bigs[dst_s]
                        nc.scalar.activation(dst[:, H2:2 * H2], til_ps[:],
                                             AF.Copy, scale=r_[:])
                        # diff/prod: all-SBUF f16, split halves DVE/Pool
                        nc.vector.tensor_sub(dst[:, 2 * H2:2 * H2 + 512],
                                             dst[:, 0:512], dst[:, H2:H2 + 512])
                        nc.gpsimd.tensor_sub(dst[:, 2 * H2 + 512:3 * H2],
                                             dst[:, 512:H2],
                                             dst[:, H2 + 512:2 * H2])
                        nc.gpsimd.tensor_mul(dst[:, 3 * H2:3 * H2 + 512],
                                             dst[:, 0:512], dst[:, H2:H2 + 512])
                        nc.vector.tensor_mul(dst[:, 3 * H2 + 512:4 * H2],
                                             dst[:, 512:H2],
                                             dst[:, H2 + 512:2 * H2])
                    # first half (bar|til) ships while diff/prod compute
                    nc.sync.dma_start(outA_d.ap()[n, :, 0:2 * H2],
                                      bigs["a"][:, 0:2 * H2])
                    nc.sync.dma_start(outB_d.ap()[n, :, 0:2 * H2],
                                      bigs["b"][:, 0:2 * H2])
                    nc.sync.dma_start(outA_d.ap()[n, :, 2 * H2:OUT],
                                      bigs["a"][:, 2 * H2:OUT])
                    nc.sync.dma_start(outB_d.ap()[n, :, 2 * H2:OUT],
                                      bigs["b"][:, 2 * H2:OUT])

    nc.compile()
    return nc


def _get_nc():
    if "nc" not in _CACHE:
        _CACHE["nc"] = _build()
    return _CACHE["nc"]


def prep_in_maps(inputs):
    A = np.asarray(inputs["A"])
    B = np.asarray(inputs["B"])
    embed = np.asarray(inputs["embed"], dtype=np.float32)
    # gates stay in pytorch order [i,f,g,o]: fc 0-7 feed the IF psum bank,
    # fc 8-15 the GO bank
    perm = np.arange(4 * H)
    wmat = {}
    for d in "fb":
        suf = "_" + d
        wihT = np.asarray(inputs["Wih" + suf], dtype=np.float32)[perm].T
        bias = (np.asarray(inputs["bih" + suf], dtype=np.float32)
                + np.asarray(inputs["bhh" + suf], dtype=np.float32))[perm]
        wihT_aug = np.concatenate([wihT, bias[None, :]], axis=0)  # [301, 2048]
        whhT = np.asarray(inputs["Whh" + suf], dtype=np.float32)[perm].T
        wmat[d] = (np.ascontiguousarray(wihT_aug, dtype=np.float16),
                   np.ascontiguousarray(whhT, dtype=np.float16))

    xa = embed[A]    # [BSZ, T, E]
    xb = embed[B]

    in_maps = []
    for c in range(NCORES):
        sl = slice(PB * c, PB * (c + 1))
        xc = np.concatenate([xa[sl], xb[sl]], axis=0)          # [RW, T, E]
        xT = xc.transpose(2, 1, 0)                             # [E, T, RW]
        xTb = np.concatenate(
            [xT, np.ones((1, T, RW), np.float32)], axis=0)     # [EB, T, RW]
        in_maps.append({
            "xTb": np.ascontiguousarray(xTb, dtype=np.float16),
            "wihT_f": wmat["f"][0], "whhT_f": wmat["f"][1],
            "wihT_b": wmat["b"][0], "whhT_b": wmat["b"][1],
        })
    return in_maps


def kernel(**inputs):
    from concourse.bass_utils import run_bass_kernel_spmd

    in_maps = prep_in_maps(inputs)
    nc = _get_nc()
    res = run_bass_kernel_spmd(nc, in_maps, core_ids=list(range(NCORES)))
    outA = np.concatenate(
        [res.results[c]["outA"] for c in range(NCORES)], axis=0)
    outB = np.concatenate(
        [res.results[c]["outB"] for c in range(NCORES)], axis=0)
    return outA.astype(np.float32), outB.astype(np.float32)


# revision 48
# speedup vs baseline: 3.5374x; 1.0026x over previous
"""Bass/Trainium2 kernel for nn_Encoder (embedding -> BiLSTM -> cross attention
-> enhancement).

Sharding: data-parallel over batch, 16 items per core on 8 NeuronCores (no
collectives). Per core the A and B sequences are stacked into RW=32 rows.

The BiLSTM scan runs FEATURE-MAJOR: all state (gates, c, h) lives in
[128-feature-partition x 32-row] tiles, so the recurrence needs no PE
transposes and every elementwise op uses all 128 partitions. The input
projection x@Wih^T + bias is fused into the gate PSUM accumulation as extra
matmuls against a bias-augmented [301, 2048] weight (xT carries a constant
ones row), issued one step ahead so they fill PE gaps while the current
step's elementwise chain runs. Gates accumulate into TWO psum bank-tiles
per (dir, step) - [i|f] and [g|o] in pytorch gate order - so sigmoid(i,f)
starts after only half the Whh matmuls. ACT writes activated gates to
packed f16 SBUF tiles, putting every cell-update DVE op on the 2-byte
fast path; h is written (in two halves, unblocking the next step's first
Whh matmuls early) as f16 into a persistent SBUF buffer that both the
next step's matmuls and the attention phase consume.

Attention: per item, PE transposes build time-major [T, H2] "bar" tiles from
the SBUF h buffer, E/E^T come from feature-major matmuls, softmax via
Exp-with-accum, soft alignment matmuls, and the 4-way enhancement concat is
assembled in one [128, 4096] f16 tile and written with a single DMA per
sequence. Outputs are f16 in DRAM; the host converts to f32.
"""

import numpy as np

V, E, H = 32000, 300, 512
BSZ, T = 128, 128
NCORES = 8
PB = BSZ // NCORES          # 16 batch items per core
RW = 2 * PB                 # 32 stacked rows (A items then B items)
G4 = 4 * H                  # 2048 gate width
H2 = 2 * H                  # 1024 bilstm output width
EB = E + 1                  # embedding dim + folded bias row
KCH = [(0, 128), (128, 128), (256, EB - 256)]   # chunks of EB=301
OUT = 4 * H2

_CACHE = {}


def _build():
    import concourse.mybir as mybir
    import concourse.tile as tile
    from concourse import bacc
    from concourse.masks import make_identity

    F32 = mybir.dt.float32
    F16 = mybir.dt.float16
    AF = mybir.ActivationFunctionType
    ALU = mybir.AluOpType
    AX = mybir.AxisListType

    nc = bacc.Bacc("TRN2", target_bir_lowering=False, debug=False,
                   num_devices=NCORES)

    xT_d = nc.dram_tensor("xTb", [EB, T, RW], F16, kind="ExternalInput")
    wih_d = {d: nc.dram_tensor(f"wihT_{d}", [EB, G4], F16, kind="ExternalInput")
             for d in "fb"}
    whh_d = {d: nc.dram_tensor(f"whhT_{d}", [H, G4], F16, kind="ExternalInput")
             for d in "fb"}
    outA_d = nc.dram_tensor("outA", [PB, T, OUT], F16, kind="ExternalOutput")
    outB_d = nc.dram_tensor("outB", [PB, T, OUT], F16, kind="ExternalOutput")

    # gates split across two psum bank-tiles: IF=[i|f], GO=[g|o]; weights
    # keep pytorch gate order [i,f,g,o] so fc 0-7 -> IF bank, 8-15 -> GO
    SI = slice(0, 128)
    SF = slice(128, 256)
    SG = slice(0, 128)
    SO = slice(128, 256)

    with tile.TileContext(nc) as tc:
        with tc.tile_pool(name="const", bufs=1) as const, \
             tc.tile_pool(name="hbuf", bufs=1) as hbuf:
            identf = const.tile([128, 128], F32)
            make_identity(nc, identf[:])
            ident16 = const.tile([128, 128], F16)
            nc.vector.tensor_copy(ident16[:], identf[:])
            # persistent h buffers: [feature-in-chunk, t, kc, row], f16
            Hb = {d: hbuf.tile([128, T, 4, RW], F16, name=f"Hb_{d}")
                  for d in "fb"}

            # ---------------- Phase 1+2: fused proj + scan ----------------
            with tc.tile_pool(name="wst", bufs=1) as wst, \
                 tc.tile_pool(name="sst", bufs=1) as sst, \
                 tc.tile_pool(name="ew", bufs=2) as ew, \
                 tc.tile_pool(name="gps", bufs=2, space="PSUM") as gps_pool:
                xT_sb = []
                qs = [nc.sync, nc.scalar, nc.gpsimd]
                qi = 0
                for ki, (ko, ks) in enumerate(KCH):
                    t_ = wst.tile([ks, T, RW], F16, tag=f"xT{ki}")
                    qs[qi % 3].dma_start(t_[:], xT_d.ap()[ko:ko + ks])
                    qi += 1
                    xT_sb.append(t_)
                wih_sb, whh_sb = {}, {}
                for d in "fb":
                    wih_sb[d] = []
                    for ki, (ko, ks) in enumerate(KCH):
                        w = wst.tile([ks, G4], F16, tag=f"wih{d}{ki}")
                        qs[qi % 3].dma_start(w[:], wih_d[d].ap()[ko:ko + ks])
                        qi += 1
                        wih_sb[d].append(w)
                    whh_sb[d] = []
                    for kr in range(4):
                        w = wst.tile([128, G4], F16, tag=f"whh{d}{kr}")
                        qs[qi % 3].dma_start(
                            w[:], whh_d[d].ap()[kr * 128:(kr + 1) * 128])
                        qi += 1
                        whh_sb[d].append(w)
                # c state in f16: every cell-update DVE op is then a packed
                # 2-byte all-SBUF op (half-rate cycles on DVE)
                c_st = {d: sst.tile([128, 128], F16, name=f"c_{d}")
                        for d in "fb"}

                def x_accum(t, d):
                    """Issue x-part (incl bias) matmuls for step t into two
                    fresh psum bank-tiles (IF gates, GO gates). One
                    accumulation group per bank: start=True only on the
                    bank's first matmul, stop=True only on its last (at t==0
                    the last x matmul, else the last Whh matmul)."""
                    tx = t if d == "f" else T - 1 - t
                    gif = gps_pool.tile([128, 512], F32, tag=f"gi{d}",
                                        name=f"gi{d}")
                    ggo = gps_pool.tile([128, 512], F32, tag=f"gg{d}",
                                        name=f"gg{d}")
                    for bank, g in ((0, gif), (1, ggo)):
                        for f8 in range(8):
                            fc = bank * 8 + f8
                            fs = slice(fc * 128, (fc + 1) * 128)
                            for ki in range(3):
                                nc.tensor.matmul(
                                    g[:, f8 * RW:(f8 + 1) * RW],
                                    wih_sb[d][ki][:, fs],
                                    xT_sb[ki][:, tx, :],
                                    start=(f8 == 0 and ki == 0),
                                    stop=(t == 0 and f8 == 7 and ki == 2))
                    return gif, ggo

                gcur = {d: x_accum(0, d) for d in "fb"}

                for t in range(T):
                    for d in "fb":
                        tx = t if d == "f" else T - 1 - t
                        txp = tx - 1 if d == "f" else tx + 1
                        gif, ggo = gcur[d]
                        if t > 0:
                            for bank, g in ((0, gif), (1, ggo)):
                                for f8 in range(8):
                                    fc = bank * 8 + f8
                                    fs = slice(fc * 128, (fc + 1) * 128)
                                    for kr in range(4):
                                        nc.tensor.matmul(
                                            g[:, f8 * RW:(f8 + 1) * RW],
                                            whh_sb[d][kr][:, fs],
                                            Hb[d][:, txp, kr, :],
                                            start=False,
                                            stop=(f8 == 7 and kr == 3))
                        # elementwise, all [128, 128] feature-major. ACT
                        # moves activated gates PSUM -> packed f16 SBUF so
                        # every DVE op runs the 2-byte fast path; DVE also
                        # never reads two PSUM operands this way.
                        cs = c_st[d]
                        sg = ew.tile([128, 256], F16, tag=f"sg{d}")
                        nc.scalar.activation(sg[:], gif[:, 0:256], AF.Sigmoid)
                        gp = ew.tile([128, 128], F16, tag=f"gp{d}")
                        nc.scalar.activation(gp[:], ggo[:, SG], AF.Tanh)
                        if t == 0:
                            nc.vector.tensor_mul(cs[:], sg[:, 0:128], gp[:])
                        else:
                            cq = ew.tile([128, 128], F16, tag=f"cq{d}")
                            nc.vector.tensor_mul(cq[:], sg[:, 128:256], cs[:])
                            pp = ew.tile([128, 128], F16, tag=f"pp{d}")
                            nc.vector.tensor_mul(pp[:], sg[:, 0:128], gp[:])
                            nc.vector.tensor_add(cs[:], pp[:], cq[:])
                        so = ew.tile([128, 128], F16, tag=f"so{d}")
                        nc.scalar.activation(so[:], ggo[:, SO], AF.Sigmoid)
                        tc_ = ew.tile([128, 128], F16, tag=f"tc{d}")
                        nc.scalar.activation(tc_[:], cs[:], AF.Tanh)
                        # write h in two halves so the next step's first Whh
                        # matmuls (kr 0,1) can start before the second half
                        nc.vector.tensor_mul(Hb[d][:, tx, 0:2, :],
                                             so[:, 0:64], tc_[:, 0:64])
                        nc.vector.tensor_mul(Hb[d][:, tx, 2:4, :],
                                             so[:, 64:128], tc_[:, 64:128])
                        if t + 1 < T:
                            gcur[d] = x_accum(t + 1, d)

            # ---------------- Phase 3: attention + enhancement ----------------
            with tc.tile_pool(name="abuf", bufs=3) as abuf, \
                 tc.tile_pool(name="zbuf", bufs=2) as zbuf, \
                 tc.tile_pool(name="tmps", bufs=1, space="PSUM") as tmps, \
                 tc.tile_pool(name="eps", bufs=1, space="PSUM") as eps_pool, \
                 tc.tile_pool(name="tilps", bufs=1, space="PSUM") as til_pool:
                for n in range(PB):
                    bigs = {}
                    for s, row in (("a", n), ("b", PB + n)):
                        big = abuf.tile([128, OUT], F16, tag=f"big{s}")
                        tm_ps = tmps.tile([128, H2], F16, tag="tm")
                        for c8 in range(8):
                            d, kc = "fb"[c8 // 4], c8 % 4
                            nc.tensor.transpose(
                                tm_ps[:, c8 * 128:(c8 + 1) * 128],
                                Hb[d][:, :, kc, row], ident16[:])
                        nc.vector.tensor_copy(big[:, 0:H2], tm_ps[:])
                        bigs[s] = big
                    e1t = eps_pool.tile([128, 128], F32, tag="e1")
                    e2t = eps_pool.tile([128, 128], F32, tag="e2")
                    e_ps = e1t[:]
                    e2_ps = e2t[:]
                    for c8 in range(8):
                        d, kc = "fb"[c8 // 4], c8 % 4
                        asl = Hb[d][:, :, kc, n]
                        bsl = Hb[d][:, :, kc, PB + n]
                        nc.tensor.matmul(e_ps, asl, bsl,
                                         start=(c8 == 0), stop=(c8 == 7))
                    for c8 in range(8):
                        d, kc = "fb"[c8 // 4], c8 % 4
                        asl = Hb[d][:, :, kc, n]
                        bsl = Hb[d][:, :, kc, PB + n]
                        nc.tensor.matmul(e2_ps, bsl, asl,
                                         start=(c8 == 0), stop=(c8 == 7))
                    for ei, (ep, rhs_s, dst_s) in enumerate(
                            ((e_ps, "b", "a"), (e2_ps, "a", "b"))):
                        m_ = zbuf.tile([128, 1], F32, tag=f"m{ei}")
                        nc.vector.tensor_reduce(m_[:], ep, axis=AX.X,
                                                op=ALU.max, negate=True)
                        z_ = zbuf.tile([128, 128], F16, tag=f"z{ei}")
                        s_ = zbuf.tile([128, 1], F32, tag=f"s{ei}")
                        nc.scalar.activation(z_[:], ep, AF.Exp, bias=m_[:],
                                             accum_out=s_[:])
                        r_ = zbuf.tile([128, 1], F32, tag=f"r{ei}")
                        nc.vector.reciprocal(r_[:], s_[:])
                        ztp = eps_pool.tile([128, 128], F16, tag="ztp")
                        nc.tensor.transpose(ztp[:], z_[:], ident16[:])
                        zt = zbuf.tile([128, 128], F16, tag=f"zt{ei}")
                        nc.vector.tensor_copy(zt[:], ztp[:])
                        til_ps = til_pool.tile([128, H2], F32, tag=f"til{ei}")
                        rhs = bigs[rhs_s]
                        for hh in range(2):
                            sl = slice(512 * hh, 512 * (hh + 1))
                            nc.tensor.matmul(til_ps[:, sl], zt[:], rhs[:, sl],
                                             start=True, stop=True)
                        dst = bigs[dst_s]
                        nc.scalar.activation(dst[:, H2:2 * H2], til_ps[:],
                                             AF.Copy, scale=r_[:])
                        # diff/prod: all-SBUF f16, split halves DVE/Pool
                        nc.vector.tensor_sub(dst[:, 2 * H2:2 * H2 + 512],
                                             dst[:, 0:512], dst[:, H2:H2 + 512])
                        nc.gpsimd.tensor_sub(dst[:, 2 * H2 + 512:3 * H2],
                                             dst[:, 512:H2],
                                             dst[:, H2 + 512:2 * H2])
                        nc.gpsimd.tensor_mul(dst[:, 3 * H2:3 * H2 + 512],
                                             dst[:, 0:512], dst[:, H2:H2 + 512])
                        nc.vector.tensor_mul(dst[:, 3 * H2 + 512:4 * H2],
                                             dst[:, 512:H2],
                                             dst[:, H2 + 512:2 * H2])
                    # ship sections as soon as they exist: bar right after
                    # the time-major copy, til after normalization, then
                    # diff|prod
                    for sec_lo, sec_hi in ((0, H2), (H2, 2 * H2),
                                           (2 * H2, OUT)):
                        nc.sync.dma_start(outA_d.ap()[n, :, sec_lo:sec_hi],
                                          bigs["a"][:, sec_lo:sec_hi])
                        nc.sync.dma_start(outB_d.ap()[n, :, sec_lo:sec_hi],
                                          bigs["b"][:, sec_lo:sec_hi])

    nc.compile()
    return nc


def _get_nc():
    if "nc" not in _CACHE:
        _CACHE["nc"] = _build()
    return _CACHE["nc"]


def prep_in_maps(inputs):
    A = np.asarray(inputs["A"])
    B = np.asarray(inputs["B"])
    embed = np.asarray(inputs["embed"], dtype=np.float32)
    # gates stay in pytorch order [i,f,g,o]: fc 0-7 feed the IF psum bank,
    # fc 8-15 the GO bank
    perm = np.arange(4 * H)
    wmat = {}
    for d in "fb":
        suf = "_" + d
        wihT = np.asarray(inputs["Wih" + suf], dtype=np.float32)[perm].T
        bias = (np.asarray(inputs["bih" + suf], dtype=np.float32)
                + np.asarray(inputs["bhh" + suf], dtype=np.float32))[perm]
        wihT_aug = np.concatenate([wihT, bias[None, :]], axis=0)  # [301, 2048]
        whhT = np.asarray(inputs["Whh" + suf], dtype=np.float32)[perm].T
        wmat[d] = (np.ascontiguousarray(wihT_aug, dtype=np.float16),
                   np.ascontiguousarray(whhT, dtype=np.float16))

    xa = embed[A]    # [BSZ, T, E]
    xb = embed[B]

    in_maps = []
    for c in range(NCORES):
        sl = slice(PB * c, PB * (c + 1))
        xc = np.concatenate([xa[sl], xb[sl]], axis=0)          # [RW, T, E]
        xT = xc.transpose(2, 1, 0)                             # [E, T, RW]
        xTb = np.concatenate(
            [xT, np.ones((1, T, RW), np.float32)], axis=0)     # [EB, T, RW]
        in_maps.append({
            "xTb": np.ascontiguousarray(xTb, dtype=np.float16),
            "wihT_f": wmat["f"][0], "whhT_f": wmat["f"][1],
            "wihT_b": wmat["b"][0], "whhT_b": wmat["b"][1],
        })
    return in_maps


def kernel(**inputs):
    from concourse.bass_utils import run_bass_kernel_spmd

    in_maps = prep_in_maps(inputs)
    nc = _get_nc()
    res = run_bass_kernel_spmd(nc, in_maps, core_ids=list(range(NCORES)))
    outA = np.concatenate(
        [res.results[c]["outA"] for c in range(NCORES)], axis=0)
    outB = np.concatenate(
        [res.results[c]["outB"] for c in range(NCORES)], axis=0)
    return outA.astype(np.float32), outB.astype(np.float32)
